# revision 56
# baseline (speedup 1.0000x reference)
"""InteractionNetwork (GNN message passing) Bass kernel for 8 Trainium2 cores.

Strategy (edge-sharded, per sharding hint):
  - The rr/rs inputs are one-hot by construction, so the host extracts the
    receiver/sender indices exactly (one sgemv with an arange vector each)
    and ships ONE packed f16 buffer per core (~330KB: a 1/8 shard of
    obj+weights, edge indices, and the ra.T slice) instead of the 512MB
    dense one-hot matrices — ~2.7MB total on the wire per call.
  - On device, obj and the MLP weights are reassembled from the shards with
    two AllGathers (staged through SBUF into DRAM pool tiles, since
    collectives cannot read IO tensors), so the host uploads them once.
  - Each core handles 4096 edges: node features are gathered with indirect
    DMA by index; the 4-layer relation MLP runs feature-major on the PE in
    f16 (f32 PSUM); for the rr.T @ e aggregation the one-hot receiver rows
    are rebuilt on-device (is_equal against an iota) and used as the moving
    operand of an accumulating matmul into a pinned PSUM e_agg.T
    accumulator.
  - Partial e_agg is AllReduce-summed in f32 across the 8 cores; every core
    runs the small object MLP on all 2048 nodes; host fetches core 0's
    output shard only.
  - The jitted shard_map executable is built once and cached, so warm calls
    only pay input upload + execution.
"""

import os
import sys

import numpy as np

os.environ.setdefault("MYCRO_LOCAL_CACHE", "1")
for _p in ("/opt/trn_rl_repo",):
    if os.path.isdir(_p) and _p not in sys.path:
        sys.path.insert(0, _p)

import concourse.bacc as bacc
import concourse.bass as bass
import concourse.mybir as mybir
import concourse.tile as tile
from concourse.bass_utils import run_bass_kernel_spmd
from concourse.masks import make_identity

P = 128
F32 = mybir.dt.float32
F16 = mybir.dt.float16
I32 = mybir.dt.int32
I16 = mybir.dt.int16
AF = mybir.ActivationFunctionType
ALU = mybir.AluOpType

N_OBJ, N_REL = 2048, 32768
D_OBJ, D_REL, D_EFF = 64, 32, 64
H_REL, H_OBJ = 128, 128
D_OUT = 3
N_CORES = 8

EPC = N_REL // N_CORES        # 4096 edges per core
NCH = EPC // P                # 32 chunks of 128 edges
OBJ_SH_R = N_OBJ // N_CORES   # 256 obj rows per core

# ---- packed weight blob layout (f16 elements) -----------------------------
_W_PIECES = [
    # (name, rows, cols) in packing order; loaded as [rows, cols]
    ("w1ab", P, H_REL),          # rm_w1[0:128]
    ("w1c", D_REL, H_REL),       # rm_w1[128:160]
    ("b1", H_REL, 1),
    ("w2", H_REL, H_REL),
    ("b2", H_REL, 1),
    ("w3", H_REL, H_REL),
    ("b3", H_REL, 1),
    ("w4", H_REL, D_EFF),
    ("b4", D_EFF, 1),
    ("ow1a", D_OBJ, H_OBJ),      # om_w1[0:64]
    ("ow1b", D_EFF, H_OBJ),      # om_w1[64:128]
    ("ob1", H_OBJ, 1),
    ("ow2", H_OBJ, D_OUT),
    ("ob2", D_OUT, 1),
]
_W_OFF = {}
_off = 0
for _nm, _r, _c in _W_PIECES:
    _W_OFF[_nm] = _off
    _off += _r * _c
W_TOTAL = _off                                  # 78787
W_PAD = 78848                                   # aligned pad

# ---- input blob layouts (f16 elements) ------------------------------------
# cshard: per-core 1/8 shard of (obj | weights); the device AllGathers the
#   full copies so the host uploads obj+weights once, not 8x.
# eblob: per-core edge shard: receiver/sender indices + ra.T slice
OBJ_SH = N_OBJ * D_OBJ // N_CORES               # 16384
W_SH = W_PAD // N_CORES                         # 9856
CSH = OBJ_SH + W_SH                             # per-core const-shard elems
O_IR = CSH
O_IS = O_IR + EPC
O_RA = O_IS + EPC
SSZ = O_RA + D_REL * EPC                        # one packed shard per core


def build(n_cores=N_CORES, use_collective=True):
    EG = 512                  # edges per MLP group
    T = EG // P               # 128-edge chunks per group
    n_groups = EPC // EG
    NQ = 512                  # node chunk (psum bank) for wide matmuls
    n_nq = N_OBJ // NQ
    n_obj = N_OBJ

    nc = bacc.Bacc(
        "TRN2",
        target_bir_lowering=False,
        debug=False,
        enable_asserts=False,
        num_devices=n_cores,
    )

    shard = nc.dram_tensor("shard", [SSZ], F16, kind="ExternalInput")
    pT_d = nc.dram_tensor("pT", [D_OUT, n_obj], F16, kind="ExternalOutput")

    with tile.TileContext(nc) as tc:
        with (
            tc.tile_pool(name="const", bufs=1) as const,
            tc.tile_pool(name="stream", bufs=8) as sp,
            tc.tile_pool(name="gat", bufs=4) as gp,
            tc.tile_pool(name="ec", bufs=8) as ecp,
            tc.tile_pool(name="aggp", bufs=1, space="PSUM") as aggp,
            tc.tile_pool(name="psp", bufs=4, space="PSUM") as psp,
            tc.tile_pool(name="dram", bufs=1, space="DRAM") as dp,
        ):
            # assemble full obj + weights from the per-core shards.
            # collectives cannot read IO tensors, so bounce the shard
            # through SBUF into a DRAM pool tile first.
            CCOL = CSH // P
            objall_d = dp.tile([n_obj * D_OBJ], F16)
            wall_d = dp.tile([W_PAD], F16)
            cstage = dp.tile([CSH], F16)
            with tc.tile_pool(name="stage", bufs=1) as stp:
                cs_sb = stp.tile([P, CCOL], F16)
                nc.sync.dma_start(
                    cs_sb[:], shard[0:CSH].rearrange("(p c) -> p c", c=CCOL)
                )
                nc.sync.dma_start(
                    cstage[0:CSH].rearrange("(p c) -> p c", c=CCOL), cs_sb[:]
                )
            if use_collective:
                nc.gpsimd.collective_compute(
                    "AllGather",
                    ALU.bypass,
                    replica_groups=[list(range(n_cores))],
                    ins=[cstage[0:OBJ_SH].opt()],
                    outs=[objall_d.opt()],
                )
                nc.gpsimd.collective_compute(
                    "AllGather",
                    ALU.bypass,
                    replica_groups=[list(range(n_cores))],
                    ins=[cstage[OBJ_SH : OBJ_SH + W_SH].opt()],
                    outs=[wall_d.opt()],
                )
            else:
                nc.sync.dma_start(objall_d[0:OBJ_SH], cstage[0:OBJ_SH])
                nc.sync.dma_start(
                    wall_d[0:W_SH], cstage[OBJ_SH : OBJ_SH + W_SH]
                )
            obj2d = objall_d[:].rearrange("(n d) -> n d", d=D_OBJ)

            # ---- constants -------------------------------------------------
            ident32 = const.tile([P, P], F32)
            make_identity(nc, ident32[:])
            ident16 = const.tile([P, P], F16)
            make_identity(nc, ident16[:])

            iota_i = const.tile([P, n_obj], I16)
            nc.gpsimd.iota(iota_i[:], pattern=[[1, n_obj]], base=0, channel_multiplier=0)
            iota16 = const.tile([P, n_obj], F16)
            nc.vector.tensor_copy(iota16[:], iota_i[:])

            def wmat(nm, r, c):
                # NB: explicit per-weight tag — a shared tag would make all
                # weight tiles rotate through one bufs=1 slot and deadlock
                # (slot release waits on the last MLP group).
                t = const.tile([r, c], F16, tag=f"w_{nm}")
                o = _W_OFF[nm]
                nc.sync.dma_start(
                    t[:], wall_d[o : o + r * c].rearrange("(k m) -> k m", m=c)
                )
                return t

            def wcol(nm, r):
                th = const.tile([r, 1], F16, tag=f"bh_{nm}")
                o = _W_OFF[nm]
                nc.sync.dma_start(
                    th[:], wall_d[o : o + r].rearrange("(k m) -> k m", m=1)
                )
                t = const.tile([r, 1], F32, tag=f"b_{nm}")
                nc.vector.tensor_copy(t[:], th[:])
                return t

            w1ab = wmat("w1ab", P, H_REL)
            w1c = wmat("w1c", D_REL, H_REL)
            w2 = wmat("w2", H_REL, H_REL)
            w3 = wmat("w3", H_REL, H_REL)
            w4 = wmat("w4", H_REL, D_EFF)
            ow1a = wmat("ow1a", D_OBJ, H_OBJ)
            ow1b = wmat("ow1b", D_EFF, H_OBJ)
            ow2 = wmat("ow2", H_OBJ, D_OUT)
            b1t = wcol("b1", H_REL)
            b2t = wcol("b2", H_REL)
            b3t = wcol("b3", H_REL)
            b4t = wcol("b4", D_EFF)
            ob1t = wcol("ob1", H_OBJ)
            ob2t = wcol("ob2", D_OUT)

            # edge indices: f16 (one-hot rebuild), f32 -> i32 (indirect DMA)
            idxr_h = const.tile([P, NCH], F16)
            nc.sync.dma_start(
                idxr_h[:], shard[O_IR : O_IR + EPC].rearrange("(p c) -> p c", c=NCH)
            )
            idxs_h = const.tile([P, NCH], F16)
            nc.sync.dma_start(
                idxs_h[:], shard[O_IS : O_IS + EPC].rearrange("(p c) -> p c", c=NCH)
            )
            idxr_f = const.tile([P, NCH], F32)
            nc.vector.tensor_copy(idxr_f[:], idxr_h[:])
            idxs_f = const.tile([P, NCH], F32)
            nc.vector.tensor_copy(idxs_f[:], idxs_h[:])
            idxr_i = const.tile([P, NCH], I32)
            nc.vector.tensor_copy(idxr_i[:], idxr_f[:])
            idxs_i = const.tile([P, NCH], I32)
            nc.vector.tensor_copy(idxs_i[:], idxs_f[:])

            raT = const.tile([D_REL, EPC], F16)
            nc.sync.dma_start(
                raT[:], shard[O_RA : O_RA + D_REL * EPC].rearrange("(d e) -> d e", e=EPC)
            )

            # obj.T in SBUF (for the node-model MLP)
            objT = const.tile([D_OBJ, n_obj], F16)
            for k in range(n_obj // P):
                ot = gp.tile([P, D_OBJ], F16, tag="objload")
                nc.sync.dma_start(ot[:], obj2d[k * P : (k + 1) * P, :])
                tp = psp.tile([D_OBJ, P], F16, tag="ps")
                nc.tensor.transpose(tp[:], ot[:], ident16[:])
                nc.scalar.copy(objT[:, k * P : (k + 1) * P], tp[:])

            # pinned accumulator: e_agg.T [64, n_obj] (4 PSUM banks)
            agg_ps = aggp.tile([D_EFF, n_obj], F32)

            # ---- edge phase ------------------------------------------------
            for g in range(n_groups):
                rrt = []
                b1T = sp.tile([P, EG], F16, tag="b1T")
                for t in range(T):
                    c = g * T + t
                    # one-hot receiver rows for the aggregation matmul
                    oh = sp.tile([P, n_obj], F16, tag="oh")
                    nc.vector.tensor_tensor(
                        out=oh[:],
                        in0=idxr_h[:, c : c + 1].to_broadcast([P, n_obj]),
                        in1=iota16[:],
                        op=ALU.is_equal,
                    )
                    rrt.append(oh)

                    orr_t = gp.tile([P, D_OBJ], F16, tag="gat")
                    nc.gpsimd.indirect_dma_start(
                        out=orr_t[:], out_offset=None, in_=obj2d,
                        in_offset=bass.IndirectOffsetOnAxis(
                            ap=idxr_i[:, c : c + 1], axis=0
                        ),
                    )
                    tp = psp.tile([D_OBJ, P], F16, tag="ps")
                    nc.tensor.transpose(tp[:], orr_t[:], ident16[:])
                    nc.scalar.copy(b1T[0:D_OBJ, t * P : (t + 1) * P], tp[:])

                    ors_t = gp.tile([P, D_OBJ], F16, tag="gat")
                    nc.gpsimd.indirect_dma_start(
                        out=ors_t[:], out_offset=None, in_=obj2d,
                        in_offset=bass.IndirectOffsetOnAxis(
                            ap=idxs_i[:, c : c + 1], axis=0
                        ),
                    )
                    tp2 = psp.tile([D_OBJ, P], F16, tag="ps")
                    nc.tensor.transpose(tp2[:], ors_t[:], ident16[:])
                    nc.scalar.copy(b1T[D_OBJ : 2 * D_OBJ, t * P : (t + 1) * P], tp2[:])

                # relation MLP, feature-major [features, EG]
                h1p = psp.tile([H_REL, EG], F32, tag="ps")
                nc.tensor.matmul(h1p[:], w1ab[:], b1T[:], start=True, stop=False)
                nc.tensor.matmul(
                    h1p[:], w1c[:], raT[:, g * EG : (g + 1) * EG],
                    start=False, stop=True,
                )
                h1T = sp.tile([H_REL, EG], F16, tag="hT")
                nc.scalar.activation(h1T[:], h1p[:], AF.Relu, bias=b1t[:])

                h2p = psp.tile([H_REL, EG], F32, tag="ps")
                nc.tensor.matmul(h2p[:], w2[:], h1T[:], start=True, stop=True)
                h2T = sp.tile([H_REL, EG], F16, tag="hT")
                nc.scalar.activation(h2T[:], h2p[:], AF.Relu, bias=b2t[:])

                h3p = psp.tile([H_REL, EG], F32, tag="ps")
                nc.tensor.matmul(h3p[:], w3[:], h2T[:], start=True, stop=True)
                h3T = sp.tile([H_REL, EG], F16, tag="hT")
                nc.scalar.activation(h3T[:], h3p[:], AF.Relu, bias=b3t[:])

                h4p = psp.tile([D_EFF, EG], F32, tag="ps")
                nc.tensor.matmul(h4p[:], w4[:], h3T[:], start=True, stop=True)
                eT = sp.tile([D_EFF, EG], F16, tag="eT")
                nc.scalar.activation(eT[:], h4p[:], AF.Relu, bias=b4t[:])

                # aggregate: e_agg.T += e_chunk.T @ one_hot(idx_r)_chunk
                for t in range(T):
                    ep = psp.tile([P, D_EFF], F16, tag="ps")
                    nc.tensor.transpose(
                        ep[:], eT[:, t * P : (t + 1) * P], ident16[:D_EFF, :D_EFF]
                    )
                    ec = ecp.tile([P, D_EFF], F16, tag="ec")
                    nc.scalar.copy(ec[:], ep[:])
                    first = g == 0 and t == 0
                    last = g == n_groups - 1 and t == T - 1
                    for q in range(n_obj // NQ):
                        nc.tensor.matmul(
                            agg_ps[:, q * NQ : (q + 1) * NQ],
                            ec[:],
                            rrt[t][:, q * NQ : (q + 1) * NQ],
                            start=first,
                            stop=last,
                        )

            # ---- all-reduce e_agg across cores -----------------------------
            eagg_sb = const.tile([D_EFF, n_obj], F32)
            nc.scalar.copy(eagg_sb[:], agg_ps[:])
            cc_in = dp.tile([D_EFF, n_obj], F32)
            cc_out = dp.tile([D_EFF, n_obj], F32)
            nc.sync.dma_start(cc_in[:], eagg_sb[:])
            if use_collective:
                nc.gpsimd.collective_compute(
                    "AllReduce",
                    ALU.add,
                    replica_groups=[list(range(n_cores))],
                    ins=[cc_in.opt()],
                    outs=[cc_out.opt()],
                )
            else:
                nc.sync.dma_start(cc_out[:], cc_in[:])
            eaggT = const.tile([D_EFF, n_obj], F32)
            nc.sync.dma_start(eaggT[:], cc_out[:])
            eaggT16 = const.tile([D_EFF, n_obj], F16)
            nc.vector.tensor_copy(eaggT16[:], eaggT[:])

            # ---- node phase (object MLP) -----------------------------------
            pTt = const.tile([D_OUT, n_obj], F16)
            for q in range(n_nq):
                sl = slice(q * NQ, (q + 1) * NQ)
                cp = psp.tile([H_OBJ, NQ], F32, tag="ps")
                nc.tensor.matmul(cp[:], ow1a[:], objT[:, sl], start=True, stop=False)
                nc.tensor.matmul(cp[:], ow1b[:], eaggT16[:, sl], start=False, stop=True)
                hT = sp.tile([H_OBJ, NQ], F16, tag="hT")
                nc.scalar.activation(hT[:], cp[:], AF.Relu, bias=ob1t[:])
                pp = psp.tile([D_OUT, NQ], F32, tag="ps")
                nc.tensor.matmul(pp[:], ow2[:], hT[:], start=True, stop=True)
                nc.scalar.activation(pTt[:, sl], pp[:], AF.Identity, bias=ob2t[:])
            nc.sync.dma_start(pT_d[:, :], pTt[:])

    nc.compile()
    return nc


class _Res:
    """Minimal stand-in for BassKernelResults (no trace support)."""

    def __init__(self, results):
        self.results = results
        self.exec_time_ns = None
        self.mean_exec_time_ns = None
        self.instructions_and_trace = None
        self.profile_json = None


def _make_runner(nc, n_cores):
    """Build the jitted shard_map executable ONCE; warm calls only pay
    input upload + execution (run_bass_via_pjrt re-creates the closure and
    re-traces on every call)."""
    import jax
    from jax.experimental.shard_map import shard_map
    from jax.sharding import Mesh, PartitionSpec

    from concourse.bass2jax import (
        _bass_exec_p,
        install_neuronx_cc_hook,
        partition_id_tensor,
    )

    install_neuronx_cc_hook()

    partition_name = nc.partition_id_tensor.name if nc.partition_id_tensor else None
    dbg_name = nc.dbg_addr.name if nc.dbg_addr is not None else None

    in_names = []
    out_names = []
    out_avals = []
    out_shapes = []
    for alloc in nc.m.functions[0].allocations:
        if not isinstance(alloc, mybir.MemoryLocationSet):
            continue
        name = alloc.memorylocations[0].name
        if alloc.kind == "ExternalInput":
            if name != partition_name:
                in_names.append(name)
        elif alloc.kind == "ExternalOutput":
            shape = tuple(alloc.tensor_shape)
            dtype = mybir.dt.np(alloc.dtype)
            out_names.append(name)
            out_avals.append(jax.core.ShapedArray(shape, dtype))
            out_shapes.append((shape, dtype))
    n_params = len(in_names)
    all_names = list(in_names) + list(out_names)
    if partition_name is not None:
        all_names.append(partition_name)

    donate = tuple(range(n_params, n_params + len(out_names)))

    def _body(*args):
        operands = list(args)
        if partition_name is not None:
            operands.append(partition_id_tensor())
        outs = _bass_exec_p.bind(
            *operands,
            out_avals=tuple(out_avals),
            in_names=tuple(all_names),
            out_names=tuple(out_names),
            lowering_input_output_aliases=(),
            sim_require_finite=True,
            sim_require_nnan=True,
            nc=nc,
        )
        return tuple(outs)

    devices = jax.devices()[:n_cores]
    assert len(devices) == n_cores
    mesh = Mesh(np.asarray(devices), ("core",))
    replicated = set()  # all inputs are per-core shards
    in_specs = tuple(
        PartitionSpec() if nm in replicated else PartitionSpec("core")
        for nm in in_names
    ) + (PartitionSpec("core"),) * len(out_names)
    out_specs = (PartitionSpec("core"),) * len(out_names)
    sharded = jax.jit(
        shard_map(
            _body, mesh=mesh, in_specs=in_specs, out_specs=out_specs,
            check_rep=False,
        ),
        donate_argnums=donate,
        keep_unused=True,
    )

    zeros_cache = [
        np.zeros((n_cores * shape[0], *shape[1:]), dtype)
        for shape, dtype in out_shapes
    ]
    dbg_zero = (
        np.zeros((n_cores, 2), np.uint32) if dbg_name is not None else None
    )

    def run(named_flats):
        """named_flats: dict input-name -> already-concatenated global array
        ([n_cores * per_core_len, ...])."""
        if dbg_name is not None:
            named_flats = {**named_flats, dbg_name: dbg_zero}
        concat_in = [named_flats[nm] for nm in in_names]
        out_arrs = sharded(*concat_in, *zeros_cache)
        # fetch only core 0's shard (all cores produce the full output)
        results0 = {}
        for i, name in enumerate(out_names):
            arr = out_arrs[i]
            try:
                shard0 = np.asarray(arr.addressable_shards[0].data)
                if shard0.shape != out_shapes[i][0]:
                    shard0 = shard0.reshape(n_cores, *out_shapes[i][0])[0]
            except Exception:
                shard0 = np.asarray(arr).reshape(n_cores, *out_shapes[i][0])[0]
            results0[name] = shard0
        return [results0]

    return run


_CACHE = {}
TRACE = False  # kept for test.py compat; tracing unsupported on this setup


def _get_nc():
    if "nc" not in _CACHE:
        _CACHE["nc"] = build()
    return _CACHE["nc"]


def _pack_inputs(inputs):
    """Host-side marshalling: exact index extraction + packed f16 shards
    (preallocated buffers reused across calls)."""
    f32 = lambda k: np.asarray(inputs[k], dtype=np.float32)
    rr, rs, ra = f32("rr"), f32("rs"), f32("ra")
    obj = f32("obj")
    ar = np.arange(N_OBJ, dtype=np.float32)
    idx_r = rr @ ar  # exact: single 1.0 per row, values < 2^11
    idx_s = rs @ ar

    if "bufs" not in _CACHE:
        _CACHE["bufs"] = np.zeros((N_CORES, SSZ), np.float16)
    shard = _CACHE["bufs"]

    w_flat = np.zeros(W_PAD, np.float16)
    pieces = [
        f32("rm_w1")[0:P], f32("rm_w1")[P : P + D_REL], f32("rm_b1"),
        f32("rm_w2"), f32("rm_b2"), f32("rm_w3"), f32("rm_b3"),
        f32("rm_w4"), f32("rm_b4"),
        f32("om_w1")[0:D_OBJ], f32("om_w1")[D_OBJ : D_OBJ + D_EFF],
        f32("om_b1"), f32("om_w2"), f32("om_b2"),
    ]
    o = 0
    for p_ in pieces:
        n = p_.size
        w_flat[o : o + n] = p_.astype(np.float16).ravel()
        o += n
    assert o == W_TOTAL

    obj16 = obj.astype(np.float16).ravel()
    for c in range(N_CORES):
        shard[c, 0:OBJ_SH] = obj16[c * OBJ_SH : (c + 1) * OBJ_SH]
        shard[c, OBJ_SH : OBJ_SH + W_SH] = w_flat[c * W_SH : (c + 1) * W_SH]

    idx_r3 = idx_r.reshape(N_CORES, NCH, P)
    idx_s3 = idx_s.reshape(N_CORES, NCH, P)
    for c in range(N_CORES):
        shard[c, O_IR : O_IR + EPC] = idx_r3[c].T.astype(np.float16).ravel()
        shard[c, O_IS : O_IS + EPC] = idx_s3[c].T.astype(np.float16).ravel()
        shard[c, O_RA : O_RA + D_REL * EPC] = (
            ra[c * EPC : (c + 1) * EPC].T.astype(np.float16).ravel()
        )
    return shard


def kernel(**inputs):
    nc = _get_nc()
    shard = _pack_inputs(inputs)

    if "runner" not in _CACHE:
        try:
            _CACHE["runner"] = _make_runner(nc, N_CORES)
        except Exception as e:
            print(f"kernel: cached runner unavailable ({e!r}); "
                  f"falling back to run_bass_kernel_spmd", file=sys.stderr)
            _CACHE["runner"] = None
    runner = _CACHE["runner"]
    if runner is not None:
        results = runner({"shard": shard.reshape(-1)})
        res = _Res(results)
    else:
        in_maps = [{"shard": shard[c]} for c in range(N_CORES)]
        res = run_bass_kernel_spmd(
            nc, in_maps, core_ids=list(range(N_CORES)), trace=False
        )
    _CACHE["last_results"] = res
    return np.ascontiguousarray(res.results[0]["pT"].T.astype(np.float32))


# revision 61
# speedup vs baseline: 1.2820x; 1.2820x over previous
"""InteractionNetwork (GNN message passing) Bass kernel for 8 Trainium2 cores.

Strategy (edge-sharded, per sharding hint):
  - The rr/rs inputs are one-hot by construction, so the host extracts the
    receiver/sender indices exactly (one sgemv with an arange vector each)
    and ships ONE packed f16 buffer per core (~330KB: a 1/8 shard of
    obj+weights, edge indices, and the ra.T slice) instead of the 512MB
    dense one-hot matrices — ~2.7MB total on the wire per call.
  - On device, obj and the MLP weights are reassembled from the shards with
    two AllGathers (staged through SBUF into DRAM pool tiles, since
    collectives cannot read IO tensors), so the host uploads them once.
  - Each core handles 4096 edges: node features are gathered with indirect
    DMA by index; the 4-layer relation MLP runs feature-major on the PE in
    f16 (f32 PSUM); for the rr.T @ e aggregation the one-hot receiver rows
    are rebuilt on-device (is_equal against an iota) and used as the moving
    operand of an accumulating matmul into a pinned PSUM e_agg.T
    accumulator.
  - Partial e_agg is AllReduce-summed in f32 across the 8 cores; every core
    runs the small object MLP on all 2048 nodes; host fetches core 0's
    output shard only.
  - The jitted shard_map executable is built once and cached, so warm calls
    only pay input upload + execution.
"""

import os
import sys

import numpy as np

os.environ.setdefault("MYCRO_LOCAL_CACHE", "1")
for _p in ("/opt/trn_rl_repo",):
    if os.path.isdir(_p) and _p not in sys.path:
        sys.path.insert(0, _p)

import concourse.bacc as bacc
import concourse.bass as bass
import concourse.mybir as mybir
import concourse.tile as tile
from concourse.bass_utils import run_bass_kernel_spmd
from concourse.masks import make_identity

P = 128
F32 = mybir.dt.float32
F16 = mybir.dt.float16
I32 = mybir.dt.int32
I16 = mybir.dt.int16
I8 = mybir.dt.int8
AF = mybir.ActivationFunctionType
ALU = mybir.AluOpType

N_OBJ, N_REL = 2048, 32768
D_OBJ, D_REL, D_EFF = 64, 32, 64
H_REL, H_OBJ = 128, 128
D_OUT = 3
N_CORES = 8

EPC = N_REL // N_CORES        # 4096 edges per core
NCH = EPC // P                # 32 chunks of 128 edges
OBJ_SH_R = N_OBJ // N_CORES   # 256 obj rows per core

# ---- packed weight blob layout (f16 elements) -----------------------------
_W_PIECES = [
    # (name, rows, cols) in packing order; loaded as [rows, cols]
    ("w1ab", P, H_REL),          # rm_w1[0:128]
    ("w1c", D_REL, H_REL),       # rm_w1[128:160]
    ("b1", H_REL, 1),
    ("w2", H_REL, H_REL),
    ("b2", H_REL, 1),
    ("w3", H_REL, H_REL),
    ("b3", H_REL, 1),
    ("w4", H_REL, D_EFF),
    ("b4", D_EFF, 1),
    ("ow1a", D_OBJ, H_OBJ),      # om_w1[0:64]
    ("ow1b", D_EFF, H_OBJ),      # om_w1[64:128]
    ("ob1", H_OBJ, 1),
    ("ow2", H_OBJ, D_OUT),
    ("ob2", D_OUT, 1),
]
_W_OFF = {}
_off = 0
for _nm, _r, _c in _W_PIECES:
    _W_OFF[_nm] = _off
    _off += _r * _c
W_TOTAL = _off                                  # 78787
W_PAD = 78848                                   # aligned pad

# ---- input blob layouts (f16 elements) ------------------------------------
# cshard: per-core 1/8 shard of (obj | weights); the device AllGathers the
#   full copies so the host uploads obj+weights once, not 8x.
# eblob: per-core edge shard: receiver/sender indices + ra.T slice
OBJ_SH = N_OBJ * D_OBJ // N_CORES               # 16384
W_SH = W_PAD // N_CORES                         # 9856
CSH = OBJ_SH + W_SH                             # per-core const-shard elems
O_IR = CSH
O_IS = O_IR + EPC
O_RA = O_IS + EPC                               # ra.T as int8 (2 per f16 slot)
RA_SLOTS = D_REL * EPC // 2                     # 65536 f16 slots
SSZ = O_RA + RA_SLOTS                           # one packed shard per core


def build(n_cores=N_CORES, use_collective=True):
    EG = 512                  # edges per MLP group
    T = EG // P               # 128-edge chunks per group
    n_groups = EPC // EG
    NQ = 512                  # node chunk (psum bank) for wide matmuls
    n_nq = N_OBJ // NQ
    n_obj = N_OBJ

    nc = bacc.Bacc(
        "TRN2",
        target_bir_lowering=False,
        debug=False,
        enable_asserts=False,
        num_devices=n_cores,
    )

    shard = nc.dram_tensor("shard", [SSZ], F16, kind="ExternalInput")
    pT_d = nc.dram_tensor("pT", [D_OUT, n_obj], F16, kind="ExternalOutput")

    with tile.TileContext(nc) as tc:
        with (
            tc.tile_pool(name="const", bufs=1) as const,
            tc.tile_pool(name="stream", bufs=8) as sp,
            tc.tile_pool(name="gat", bufs=4) as gp,
            tc.tile_pool(name="ec", bufs=8) as ecp,
            tc.tile_pool(name="aggp", bufs=1, space="PSUM") as aggp,
            tc.tile_pool(name="psp", bufs=4, space="PSUM") as psp,
            tc.tile_pool(name="dram", bufs=1, space="DRAM") as dp,
        ):
            # assemble full obj + weights from the per-core shards.
            # collectives cannot read IO tensors, so bounce the shard
            # through SBUF into a DRAM pool tile first.
            CCOL = CSH // P
            objall_d = dp.tile([n_obj * D_OBJ], F16)
            wall_d = dp.tile([W_PAD], F16)
            cstage = dp.tile([CSH], F16)
            with tc.tile_pool(name="stage", bufs=1) as stp:
                cs_sb = stp.tile([P, CCOL], F16)
                nc.sync.dma_start(
                    cs_sb[:], shard[0:CSH].rearrange("(p c) -> p c", c=CCOL)
                )
                nc.sync.dma_start(
                    cstage[0:CSH].rearrange("(p c) -> p c", c=CCOL), cs_sb[:]
                )
            if use_collective:
                nc.gpsimd.collective_compute(
                    "AllGather",
                    ALU.bypass,
                    replica_groups=[list(range(n_cores))],
                    ins=[cstage[0:OBJ_SH].opt()],
                    outs=[objall_d.opt()],
                )
                nc.gpsimd.collective_compute(
                    "AllGather",
                    ALU.bypass,
                    replica_groups=[list(range(n_cores))],
                    ins=[cstage[OBJ_SH : OBJ_SH + W_SH].opt()],
                    outs=[wall_d.opt()],
                )
            else:
                nc.sync.dma_start(objall_d[0:OBJ_SH], cstage[0:OBJ_SH])
                nc.sync.dma_start(
                    wall_d[0:W_SH], cstage[OBJ_SH : OBJ_SH + W_SH]
                )
            obj2d = objall_d[:].rearrange("(n d) -> n d", d=D_OBJ)

            # ---- constants -------------------------------------------------
            ident32 = const.tile([P, P], F32)
            make_identity(nc, ident32[:])
            ident16 = const.tile([P, P], F16)
            make_identity(nc, ident16[:])

            iota_i = const.tile([P, n_obj], I16)
            nc.gpsimd.iota(iota_i[:], pattern=[[1, n_obj]], base=0, channel_multiplier=0)
            iota16 = const.tile([P, n_obj], F16)
            nc.vector.tensor_copy(iota16[:], iota_i[:])

            def wmat(nm, r, c):
                # NB: explicit per-weight tag — a shared tag would make all
                # weight tiles rotate through one bufs=1 slot and deadlock
                # (slot release waits on the last MLP group).
                t = const.tile([r, c], F16, tag=f"w_{nm}")
                o = _W_OFF[nm]
                nc.sync.dma_start(
                    t[:], wall_d[o : o + r * c].rearrange("(k m) -> k m", m=c)
                )
                return t

            def wcol(nm, r):
                th = const.tile([r, 1], F16, tag=f"bh_{nm}")
                o = _W_OFF[nm]
                nc.sync.dma_start(
                    th[:], wall_d[o : o + r].rearrange("(k m) -> k m", m=1)
                )
                t = const.tile([r, 1], F32, tag=f"b_{nm}")
                nc.vector.tensor_copy(t[:], th[:])
                return t

            w1ab = wmat("w1ab", P, H_REL)
            w1c = wmat("w1c", D_REL, H_REL)
            w2 = wmat("w2", H_REL, H_REL)
            w3 = wmat("w3", H_REL, H_REL)
            w4 = wmat("w4", H_REL, D_EFF)
            ow1a = wmat("ow1a", D_OBJ, H_OBJ)
            ow1b = wmat("ow1b", D_EFF, H_OBJ)
            ow2 = wmat("ow2", H_OBJ, D_OUT)
            b1t = wcol("b1", H_REL)
            b2t = wcol("b2", H_REL)
            b3t = wcol("b3", H_REL)
            b4t = wcol("b4", D_EFF)
            ob1t = wcol("ob1", H_OBJ)
            ob2t = wcol("ob2", D_OUT)

            # edge indices: f16 (one-hot rebuild), f32 -> i32 (indirect DMA)
            idxr_h = const.tile([P, NCH], F16)
            nc.sync.dma_start(
                idxr_h[:], shard[O_IR : O_IR + EPC].rearrange("(p c) -> p c", c=NCH)
            )
            idxs_h = const.tile([P, NCH], F16)
            nc.sync.dma_start(
                idxs_h[:], shard[O_IS : O_IS + EPC].rearrange("(p c) -> p c", c=NCH)
            )
            idxr_f = const.tile([P, NCH], F32)
            nc.vector.tensor_copy(idxr_f[:], idxr_h[:])
            idxs_f = const.tile([P, NCH], F32)
            nc.vector.tensor_copy(idxs_f[:], idxs_h[:])
            idxr_i = const.tile([P, NCH], I32)
            nc.vector.tensor_copy(idxr_i[:], idxr_f[:])
            idxs_i = const.tile([P, NCH], I32)
            nc.vector.tensor_copy(idxs_i[:], idxs_f[:])

            # ra.T arrives int8 (dequant scale folded into w1c on the host)
            ra_i8 = const.tile([D_REL, EPC], I8)
            shard_i8 = shard.bitcast(I8)
            nc.sync.dma_start(
                ra_i8[:],
                shard_i8[2 * O_RA : 2 * O_RA + D_REL * EPC].rearrange(
                    "(d e) -> d e", e=EPC
                ),
            )
            raT = const.tile([D_REL, EPC], F16)
            nc.vector.tensor_copy(raT[:], ra_i8[:])

            # obj.T in SBUF (for the node-model MLP)
            objT = const.tile([D_OBJ, n_obj], F16)
            for k in range(n_obj // P):
                ot = gp.tile([P, D_OBJ], F16, tag="objload")
                nc.sync.dma_start(ot[:], obj2d[k * P : (k + 1) * P, :])
                tp = psp.tile([D_OBJ, P], F16, tag="ps")
                nc.tensor.transpose(tp[:], ot[:], ident16[:])
                nc.scalar.copy(objT[:, k * P : (k + 1) * P], tp[:])

            # pinned accumulator: e_agg.T [64, n_obj] (4 PSUM banks)
            agg_ps = aggp.tile([D_EFF, n_obj], F32)

            # ---- edge phase ------------------------------------------------
            for g in range(n_groups):
                rrt = []
                b1T = sp.tile([P, EG], F16, tag="b1T")
                for t in range(T):
                    c = g * T + t
                    # one-hot receiver rows for the aggregation matmul
                    oh = sp.tile([P, n_obj], F16, tag="oh")
                    nc.vector.tensor_tensor(
                        out=oh[:],
                        in0=idxr_h[:, c : c + 1].to_broadcast([P, n_obj]),
                        in1=iota16[:],
                        op=ALU.is_equal,
                    )
                    rrt.append(oh)

                    orr_t = gp.tile([P, D_OBJ], F16, tag="gat")
                    nc.gpsimd.indirect_dma_start(
                        out=orr_t[:], out_offset=None, in_=obj2d,
                        in_offset=bass.IndirectOffsetOnAxis(
                            ap=idxr_i[:, c : c + 1], axis=0
                        ),
                    )
                    tp = psp.tile([D_OBJ, P], F16, tag="ps")
                    nc.tensor.transpose(tp[:], orr_t[:], ident16[:])
                    nc.scalar.copy(b1T[0:D_OBJ, t * P : (t + 1) * P], tp[:])

                    ors_t = gp.tile([P, D_OBJ], F16, tag="gat")
                    nc.gpsimd.indirect_dma_start(
                        out=ors_t[:], out_offset=None, in_=obj2d,
                        in_offset=bass.IndirectOffsetOnAxis(
                            ap=idxs_i[:, c : c + 1], axis=0
                        ),
                    )
                    tp2 = psp.tile([D_OBJ, P], F16, tag="ps")
                    nc.tensor.transpose(tp2[:], ors_t[:], ident16[:])
                    nc.scalar.copy(b1T[D_OBJ : 2 * D_OBJ, t * P : (t + 1) * P], tp2[:])

                # relation MLP, feature-major [features, EG]
                h1p = psp.tile([H_REL, EG], F32, tag="ps")
                nc.tensor.matmul(h1p[:], w1ab[:], b1T[:], start=True, stop=False)
                nc.tensor.matmul(
                    h1p[:], w1c[:], raT[:, g * EG : (g + 1) * EG],
                    start=False, stop=True,
                )
                h1T = sp.tile([H_REL, EG], F16, tag="hT")
                nc.scalar.activation(h1T[:], h1p[:], AF.Relu, bias=b1t[:])

                h2p = psp.tile([H_REL, EG], F32, tag="ps")
                nc.tensor.matmul(h2p[:], w2[:], h1T[:], start=True, stop=True)
                h2T = sp.tile([H_REL, EG], F16, tag="hT")
                nc.scalar.activation(h2T[:], h2p[:], AF.Relu, bias=b2t[:])

                h3p = psp.tile([H_REL, EG], F32, tag="ps")
                nc.tensor.matmul(h3p[:], w3[:], h2T[:], start=True, stop=True)
                h3T = sp.tile([H_REL, EG], F16, tag="hT")
                nc.scalar.activation(h3T[:], h3p[:], AF.Relu, bias=b3t[:])

                h4p = psp.tile([D_EFF, EG], F32, tag="ps")
                nc.tensor.matmul(h4p[:], w4[:], h3T[:], start=True, stop=True)
                eT = sp.tile([D_EFF, EG], F16, tag="eT")
                nc.scalar.activation(eT[:], h4p[:], AF.Relu, bias=b4t[:])

                # aggregate: e_agg.T += e_chunk.T @ one_hot(idx_r)_chunk
                for t in range(T):
                    ep = psp.tile([P, D_EFF], F16, tag="ps")
                    nc.tensor.transpose(
                        ep[:], eT[:, t * P : (t + 1) * P], ident16[:D_EFF, :D_EFF]
                    )
                    ec = ecp.tile([P, D_EFF], F16, tag="ec")
                    nc.scalar.copy(ec[:], ep[:])
                    first = g == 0 and t == 0
                    last = g == n_groups - 1 and t == T - 1
                    for q in range(n_obj // NQ):
                        nc.tensor.matmul(
                            agg_ps[:, q * NQ : (q + 1) * NQ],
                            ec[:],
                            rrt[t][:, q * NQ : (q + 1) * NQ],
                            start=first,
                            stop=last,
                        )

            # ---- all-reduce e_agg across cores -----------------------------
            eagg_sb = const.tile([D_EFF, n_obj], F32)
            nc.scalar.copy(eagg_sb[:], agg_ps[:])
            cc_in = dp.tile([D_EFF, n_obj], F32)
            cc_out = dp.tile([D_EFF, n_obj], F32)
            nc.sync.dma_start(cc_in[:], eagg_sb[:])
            if use_collective:
                nc.gpsimd.collective_compute(
                    "AllReduce",
                    ALU.add,
                    replica_groups=[list(range(n_cores))],
                    ins=[cc_in.opt()],
                    outs=[cc_out.opt()],
                )
            else:
                nc.sync.dma_start(cc_out[:], cc_in[:])
            eaggT = const.tile([D_EFF, n_obj], F32)
            nc.sync.dma_start(eaggT[:], cc_out[:])
            eaggT16 = const.tile([D_EFF, n_obj], F16)
            nc.vector.tensor_copy(eaggT16[:], eaggT[:])

            # ---- node phase (object MLP) -----------------------------------
            pTt = const.tile([D_OUT, n_obj], F16)
            for q in range(n_nq):
                sl = slice(q * NQ, (q + 1) * NQ)
                cp = psp.tile([H_OBJ, NQ], F32, tag="ps")
                nc.tensor.matmul(cp[:], ow1a[:], objT[:, sl], start=True, stop=False)
                nc.tensor.matmul(cp[:], ow1b[:], eaggT16[:, sl], start=False, stop=True)
                hT = sp.tile([H_OBJ, NQ], F16, tag="hT")
                nc.scalar.activation(hT[:], cp[:], AF.Relu, bias=ob1t[:])
                pp = psp.tile([D_OUT, NQ], F32, tag="ps")
                nc.tensor.matmul(pp[:], ow2[:], hT[:], start=True, stop=True)
                nc.scalar.activation(pTt[:, sl], pp[:], AF.Identity, bias=ob2t[:])
            nc.sync.dma_start(pT_d[:, :], pTt[:])

    nc.compile()
    return nc


class _Res:
    """Minimal stand-in for BassKernelResults (no trace support)."""

    def __init__(self, results):
        self.results = results
        self.exec_time_ns = None
        self.mean_exec_time_ns = None
        self.instructions_and_trace = None
        self.profile_json = None


def _make_runner(nc, n_cores):
    """Build the jitted shard_map executable ONCE; warm calls only pay
    input upload + execution (run_bass_via_pjrt re-creates the closure and
    re-traces on every call)."""
    import jax
    from jax.experimental.shard_map import shard_map
    from jax.sharding import Mesh, PartitionSpec

    from concourse.bass2jax import (
        _bass_exec_p,
        install_neuronx_cc_hook,
        partition_id_tensor,
    )

    install_neuronx_cc_hook()

    partition_name = nc.partition_id_tensor.name if nc.partition_id_tensor else None
    dbg_name = nc.dbg_addr.name if nc.dbg_addr is not None else None

    in_names = []
    out_names = []
    out_avals = []
    out_shapes = []
    for alloc in nc.m.functions[0].allocations:
        if not isinstance(alloc, mybir.MemoryLocationSet):
            continue
        name = alloc.memorylocations[0].name
        if alloc.kind == "ExternalInput":
            if name != partition_name:
                in_names.append(name)
        elif alloc.kind == "ExternalOutput":
            shape = tuple(alloc.tensor_shape)
            dtype = mybir.dt.np(alloc.dtype)
            out_names.append(name)
            out_avals.append(jax.core.ShapedArray(shape, dtype))
            out_shapes.append((shape, dtype))
    n_params = len(in_names)
    all_names = list(in_names) + list(out_names)
    if partition_name is not None:
        all_names.append(partition_name)

    donate = tuple(range(n_params, n_params + len(out_names)))

    def _body(*args):
        operands = list(args)
        if partition_name is not None:
            operands.append(partition_id_tensor())
        outs = _bass_exec_p.bind(
            *operands,
            out_avals=tuple(out_avals),
            in_names=tuple(all_names),
            out_names=tuple(out_names),
            lowering_input_output_aliases=(),
            sim_require_finite=True,
            sim_require_nnan=True,
            nc=nc,
        )
        return tuple(outs)

    devices = jax.devices()[:n_cores]
    assert len(devices) == n_cores
    mesh = Mesh(np.asarray(devices), ("core",))
    replicated = set()  # all inputs are per-core shards
    in_specs = tuple(
        PartitionSpec() if nm in replicated else PartitionSpec("core")
        for nm in in_names
    ) + (PartitionSpec("core"),) * len(out_names)
    out_specs = (PartitionSpec("core"),) * len(out_names)
    sharded = jax.jit(
        shard_map(
            _body, mesh=mesh, in_specs=in_specs, out_specs=out_specs,
            check_rep=False,
        ),
        donate_argnums=donate,
        keep_unused=True,
    )

    zeros_cache = [
        np.zeros((n_cores * shape[0], *shape[1:]), dtype)
        for shape, dtype in out_shapes
    ]
    dbg_zero = (
        np.zeros((n_cores, 2), np.uint32) if dbg_name is not None else None
    )

    def run(named_flats):
        """named_flats: dict input-name -> already-concatenated global array
        ([n_cores * per_core_len, ...])."""
        if dbg_name is not None:
            named_flats = {**named_flats, dbg_name: dbg_zero}
        concat_in = [named_flats[nm] for nm in in_names]
        out_arrs = sharded(*concat_in, *zeros_cache)
        # fetch only core 0's shard (all cores produce the full output)
        results0 = {}
        for i, name in enumerate(out_names):
            arr = out_arrs[i]
            try:
                shard0 = np.asarray(arr.addressable_shards[0].data)
                if shard0.shape != out_shapes[i][0]:
                    shard0 = shard0.reshape(n_cores, *out_shapes[i][0])[0]
            except Exception:
                shard0 = np.asarray(arr).reshape(n_cores, *out_shapes[i][0])[0]
            results0[name] = shard0
        return [results0]

    return run


_CACHE = {}
TRACE = False  # kept for test.py compat; tracing unsupported on this setup


def _get_nc():
    if "nc" not in _CACHE:
        _CACHE["nc"] = build()
    return _CACHE["nc"]


def _pack_inputs(inputs):
    """Host-side marshalling: exact index extraction + packed f16 shards
    (preallocated buffers reused across calls)."""
    f32 = lambda k: np.asarray(inputs[k], dtype=np.float32)
    rr, rs, ra = f32("rr"), f32("rs"), f32("ra")
    obj = f32("obj")
    ar = np.arange(N_OBJ, dtype=np.float32)
    idx_r = rr @ ar  # exact: single 1.0 per row, values < 2^11
    idx_s = rs @ ar

    if "bufs" not in _CACHE:
        _CACHE["bufs"] = np.zeros((N_CORES, SSZ), np.float16)
    shard = _CACHE["bufs"]

    # int8-quantize ra per feature; fold the dequant scale into w1c so the
    # device MLP is unchanged: ra @ w1c == q @ (diag(s) @ w1c)
    ra_s = np.maximum(np.abs(ra).max(axis=0) / 127.0, 1e-12).astype(np.float32)
    ra_q = np.rint(ra / ra_s).astype(np.int8)
    w1c_scaled = f32("rm_w1")[P : P + D_REL] * ra_s[:, None]

    w_flat = np.zeros(W_PAD, np.float16)
    pieces = [
        f32("rm_w1")[0:P], w1c_scaled, f32("rm_b1"),
        f32("rm_w2"), f32("rm_b2"), f32("rm_w3"), f32("rm_b3"),
        f32("rm_w4"), f32("rm_b4"),
        f32("om_w1")[0:D_OBJ], f32("om_w1")[D_OBJ : D_OBJ + D_EFF],
        f32("om_b1"), f32("om_w2"), f32("om_b2"),
    ]
    o = 0
    for p_ in pieces:
        n = p_.size
        w_flat[o : o + n] = p_.astype(np.float16).ravel()
        o += n
    assert o == W_TOTAL

    obj16 = obj.astype(np.float16).ravel()
    for c in range(N_CORES):
        shard[c, 0:OBJ_SH] = obj16[c * OBJ_SH : (c + 1) * OBJ_SH]
        shard[c, OBJ_SH : OBJ_SH + W_SH] = w_flat[c * W_SH : (c + 1) * W_SH]

    idx_r3 = idx_r.reshape(N_CORES, NCH, P)
    idx_s3 = idx_s.reshape(N_CORES, NCH, P)
    for c in range(N_CORES):
        shard[c, O_IR : O_IR + EPC] = idx_r3[c].T.astype(np.float16).ravel()
        shard[c, O_IS : O_IS + EPC] = idx_s3[c].T.astype(np.float16).ravel()
        shard[c, O_RA : O_RA + RA_SLOTS].view(np.int8)[...] = (
            ra_q[c * EPC : (c + 1) * EPC].T.ravel()
        )
    return shard


def kernel(**inputs):
    nc = _get_nc()
    shard = _pack_inputs(inputs)

    if "runner" not in _CACHE:
        try:
            _CACHE["runner"] = _make_runner(nc, N_CORES)
        except Exception as e:
            print(f"kernel: cached runner unavailable ({e!r}); "
                  f"falling back to run_bass_kernel_spmd", file=sys.stderr)
            _CACHE["runner"] = None
    runner = _CACHE["runner"]
    if runner is not None:
        results = runner({"shard": shard.reshape(-1)})
        res = _Res(results)
    else:
        in_maps = [{"shard": shard[c]} for c in range(N_CORES)]
        res = run_bass_kernel_spmd(
            nc, in_maps, core_ids=list(range(N_CORES)), trace=False
        )
    _CACHE["last_results"] = res
    return np.ascontiguousarray(res.results[0]["pT"].T.astype(np.float32))


# revision 63
# speedup vs baseline: 1.4011x; 1.0928x over previous
"""InteractionNetwork (GNN message passing) Bass kernel for 8 Trainium2 cores.

Strategy (edge-sharded, per sharding hint):
  - The rr/rs inputs are one-hot by construction, so the host extracts the
    receiver/sender indices exactly (one sgemv with an arange vector each)
    and ships ONE packed f16 buffer per core (~330KB: a 1/8 shard of
    obj+weights, edge indices, and the ra.T slice) instead of the 512MB
    dense one-hot matrices — ~2.7MB total on the wire per call.
  - On device, obj and the MLP weights are reassembled from the shards with
    two AllGathers (staged through SBUF into DRAM pool tiles, since
    collectives cannot read IO tensors), so the host uploads them once.
  - Each core handles 4096 edges: node features are gathered with indirect
    DMA by index; the 4-layer relation MLP runs feature-major on the PE in
    f16 (f32 PSUM); for the rr.T @ e aggregation the one-hot receiver rows
    are rebuilt on-device (is_equal against an iota) and used as the moving
    operand of an accumulating matmul into a pinned PSUM e_agg.T
    accumulator.
  - Partial e_agg is AllReduce-summed in f32 across the 8 cores; every core
    runs the small object MLP on all 2048 nodes; host fetches core 0's
    output shard only.
  - The jitted shard_map executable is built once and cached, so warm calls
    only pay input upload + execution.
"""

import os
import sys

import numpy as np

os.environ.setdefault("MYCRO_LOCAL_CACHE", "1")
for _p in ("/opt/trn_rl_repo",):
    if os.path.isdir(_p) and _p not in sys.path:
        sys.path.insert(0, _p)

import concourse.bacc as bacc
import concourse.bass as bass
import concourse.mybir as mybir
import concourse.tile as tile
from concourse.bass_utils import run_bass_kernel_spmd
from concourse.masks import make_identity

P = 128
F32 = mybir.dt.float32
F16 = mybir.dt.float16
I32 = mybir.dt.int32
I16 = mybir.dt.int16
I8 = mybir.dt.int8
AF = mybir.ActivationFunctionType
ALU = mybir.AluOpType

N_OBJ, N_REL = 2048, 32768
D_OBJ, D_REL, D_EFF = 64, 32, 64
H_REL, H_OBJ = 128, 128
D_OUT = 3
N_CORES = 8

EPC = N_REL // N_CORES        # 4096 edges per core
NCH = EPC // P                # 32 chunks of 128 edges
OBJ_SH_R = N_OBJ // N_CORES   # 256 obj rows per core

# ---- packed weight blob layout (f16 elements) -----------------------------
_W_PIECES = [
    # (name, rows, cols) in packing order; loaded as [rows, cols]
    ("w1ab", P, H_REL),          # rm_w1[0:128]
    ("w1c", D_REL, H_REL),       # rm_w1[128:160]
    ("b1", H_REL, 1),
    ("w2", H_REL, H_REL),
    ("b2", H_REL, 1),
    ("w3", H_REL, H_REL),
    ("b3", H_REL, 1),
    ("w4", H_REL, D_EFF),
    ("b4", D_EFF, 1),
    ("ow1a", D_OBJ, H_OBJ),      # om_w1[0:64]
    ("ow1b", D_EFF, H_OBJ),      # om_w1[64:128]
    ("ob1", H_OBJ, 1),
    ("ow2", H_OBJ, D_OUT),
    ("ob2", D_OUT, 1),
]
_W_OFF = {}
_off = 0
for _nm, _r, _c in _W_PIECES:
    _W_OFF[_nm] = _off
    _off += _r * _c
W_TOTAL = _off                                  # 78787
W_PAD = 78848                                   # aligned pad

# ---- input blob layouts (f16 elements) ------------------------------------
# cshard: per-core 1/8 shard of (obj | weights); the device AllGathers the
#   full copies so the host uploads obj+weights once, not 8x.
# eblob: per-core edge shard: receiver/sender indices + ra.T slice
OBJ_SH = N_OBJ * D_OBJ // N_CORES               # 16384
W_SH = W_PAD // N_CORES                         # 9856
CSH = OBJ_SH + W_SH                             # per-core const-shard elems
O_IR = CSH
O_IS = O_IR + EPC
O_RA = O_IS + EPC                               # ra.T as int8 (2 per f16 slot)
RA_SLOTS = D_REL * EPC // 2                     # 65536 f16 slots
SSZ = O_RA + RA_SLOTS                           # one packed shard per core


def build(n_cores=N_CORES, use_collective=True):
    EG = 512                  # edges per MLP group
    T = EG // P               # 128-edge chunks per group
    n_groups = EPC // EG
    NQ = 512                  # node chunk (psum bank) for wide matmuls
    n_nq = N_OBJ // NQ
    n_obj = N_OBJ

    nc = bacc.Bacc(
        "TRN2",
        target_bir_lowering=False,
        debug=False,
        enable_asserts=False,
        num_devices=n_cores,
    )

    shard = nc.dram_tensor("shard", [SSZ], F16, kind="ExternalInput")
    pT_d = nc.dram_tensor("pT", [D_OUT, n_obj], F16, kind="ExternalOutput")

    with tile.TileContext(nc) as tc:
        with (
            tc.tile_pool(name="const", bufs=1) as const,
            tc.tile_pool(name="stream", bufs=8) as sp,
            tc.tile_pool(name="gat", bufs=4) as gp,
            tc.tile_pool(name="ec", bufs=8) as ecp,
            tc.tile_pool(name="aggp", bufs=1, space="PSUM") as aggp,
            tc.tile_pool(name="psp", bufs=4, space="PSUM") as psp,
            tc.tile_pool(name="dram", bufs=1, space="DRAM") as dp,
        ):
            # assemble full obj + weights from the per-core shards.
            # collectives cannot read IO tensors, so bounce the shard
            # through SBUF into a DRAM pool tile first.
            CCOL = CSH // P
            objall_d = dp.tile([n_obj * D_OBJ], F16)
            wall_d = dp.tile([W_PAD], F16)
            cstage = dp.tile([CSH], F16)
            with tc.tile_pool(name="stage", bufs=1) as stp:
                cs_sb = stp.tile([P, CCOL], F16)
                nc.sync.dma_start(
                    cs_sb[:], shard[0:CSH].rearrange("(p c) -> p c", c=CCOL)
                )
                nc.sync.dma_start(
                    cstage[0:CSH].rearrange("(p c) -> p c", c=CCOL), cs_sb[:]
                )
            if use_collective:
                nc.gpsimd.collective_compute(
                    "AllGather",
                    ALU.bypass,
                    replica_groups=[list(range(n_cores))],
                    ins=[cstage[0:OBJ_SH].opt()],
                    outs=[objall_d.opt()],
                )
                nc.gpsimd.collective_compute(
                    "AllGather",
                    ALU.bypass,
                    replica_groups=[list(range(n_cores))],
                    ins=[cstage[OBJ_SH : OBJ_SH + W_SH].opt()],
                    outs=[wall_d.opt()],
                )
            else:
                nc.sync.dma_start(objall_d[0:OBJ_SH], cstage[0:OBJ_SH])
                nc.sync.dma_start(
                    wall_d[0:W_SH], cstage[OBJ_SH : OBJ_SH + W_SH]
                )
            obj2d = objall_d[:].rearrange("(n d) -> n d", d=D_OBJ)

            # ---- constants -------------------------------------------------
            ident32 = const.tile([P, P], F32)
            make_identity(nc, ident32[:])
            ident16 = const.tile([P, P], F16)
            make_identity(nc, ident16[:])

            iota_i = const.tile([P, n_obj], I16)
            nc.gpsimd.iota(iota_i[:], pattern=[[1, n_obj]], base=0, channel_multiplier=0)
            iota16 = const.tile([P, n_obj], F16)
            nc.vector.tensor_copy(iota16[:], iota_i[:])

            def wmat(nm, r, c):
                # NB: explicit per-weight tag — a shared tag would make all
                # weight tiles rotate through one bufs=1 slot and deadlock
                # (slot release waits on the last MLP group).
                t = const.tile([r, c], F16, tag=f"w_{nm}")
                o = _W_OFF[nm]
                nc.sync.dma_start(
                    t[:], wall_d[o : o + r * c].rearrange("(k m) -> k m", m=c)
                )
                return t

            def wcol(nm, r):
                th = const.tile([r, 1], F16, tag=f"bh_{nm}")
                o = _W_OFF[nm]
                nc.sync.dma_start(
                    th[:], wall_d[o : o + r].rearrange("(k m) -> k m", m=1)
                )
                t = const.tile([r, 1], F32, tag=f"b_{nm}")
                nc.vector.tensor_copy(t[:], th[:])
                return t

            w1ab = wmat("w1ab", P, H_REL)
            w1c = wmat("w1c", D_REL, H_REL)
            w2 = wmat("w2", H_REL, H_REL)
            w3 = wmat("w3", H_REL, H_REL)
            w4 = wmat("w4", H_REL, D_EFF)
            ow1a = wmat("ow1a", D_OBJ, H_OBJ)
            ow1b = wmat("ow1b", D_EFF, H_OBJ)
            ow2 = wmat("ow2", H_OBJ, D_OUT)
            b1t = wcol("b1", H_REL)
            b2t = wcol("b2", H_REL)
            b3t = wcol("b3", H_REL)
            b4t = wcol("b4", D_EFF)
            ob1t = wcol("ob1", H_OBJ)
            ob2t = wcol("ob2", D_OUT)

            # edge indices: f16 (one-hot rebuild), f32 -> i32 (indirect DMA)
            idxr_h = const.tile([P, NCH], F16)
            nc.sync.dma_start(
                idxr_h[:], shard[O_IR : O_IR + EPC].rearrange("(p c) -> p c", c=NCH)
            )
            idxs_h = const.tile([P, NCH], F16)
            nc.sync.dma_start(
                idxs_h[:], shard[O_IS : O_IS + EPC].rearrange("(p c) -> p c", c=NCH)
            )
            idxr_f = const.tile([P, NCH], F32)
            nc.vector.tensor_copy(idxr_f[:], idxr_h[:])
            idxs_f = const.tile([P, NCH], F32)
            nc.vector.tensor_copy(idxs_f[:], idxs_h[:])
            idxr_i = const.tile([P, NCH], I32)
            nc.vector.tensor_copy(idxr_i[:], idxr_f[:])
            idxs_i = const.tile([P, NCH], I32)
            nc.vector.tensor_copy(idxs_i[:], idxs_f[:])

            # ra.T arrives int8 (dequant scale folded into w1c on the host)
            ra_i8 = const.tile([D_REL, EPC], I8)
            shard_i8 = shard.bitcast(I8)
            nc.sync.dma_start(
                ra_i8[:],
                shard_i8[2 * O_RA : 2 * O_RA + D_REL * EPC].rearrange(
                    "(d e) -> d e", e=EPC
                ),
            )
            raT = const.tile([D_REL, EPC], F16)
            nc.vector.tensor_copy(raT[:], ra_i8[:])

            # obj.T in SBUF (for the node-model MLP)
            objT = const.tile([D_OBJ, n_obj], F16)
            for k in range(n_obj // P):
                ot = gp.tile([P, D_OBJ], F16, tag="objload")
                nc.sync.dma_start(ot[:], obj2d[k * P : (k + 1) * P, :])
                tp = psp.tile([D_OBJ, P], F16, tag="ps")
                nc.tensor.transpose(tp[:], ot[:], ident16[:])
                nc.scalar.copy(objT[:, k * P : (k + 1) * P], tp[:])

            # pinned accumulator: e_agg.T [64, n_obj] (4 PSUM banks)
            agg_ps = aggp.tile([D_EFF, n_obj], F32)

            # ---- edge phase ------------------------------------------------
            for g in range(n_groups):
                rrt = []
                b1T = sp.tile([P, EG], F16, tag="b1T")
                for t in range(T):
                    c = g * T + t
                    # one-hot receiver rows for the aggregation matmul
                    oh = sp.tile([P, n_obj], F16, tag="oh")
                    nc.vector.tensor_tensor(
                        out=oh[:],
                        in0=idxr_h[:, c : c + 1].to_broadcast([P, n_obj]),
                        in1=iota16[:],
                        op=ALU.is_equal,
                    )
                    rrt.append(oh)

                    orr_t = gp.tile([P, D_OBJ], F16, tag="gat")
                    nc.gpsimd.indirect_dma_start(
                        out=orr_t[:], out_offset=None, in_=obj2d,
                        in_offset=bass.IndirectOffsetOnAxis(
                            ap=idxr_i[:, c : c + 1], axis=0
                        ),
                    )
                    tp = psp.tile([D_OBJ, P], F16, tag="ps")
                    nc.tensor.transpose(tp[:], orr_t[:], ident16[:])
                    nc.scalar.copy(b1T[0:D_OBJ, t * P : (t + 1) * P], tp[:])

                    ors_t = gp.tile([P, D_OBJ], F16, tag="gat")
                    nc.gpsimd.indirect_dma_start(
                        out=ors_t[:], out_offset=None, in_=obj2d,
                        in_offset=bass.IndirectOffsetOnAxis(
                            ap=idxs_i[:, c : c + 1], axis=0
                        ),
                    )
                    tp2 = psp.tile([D_OBJ, P], F16, tag="ps")
                    nc.tensor.transpose(tp2[:], ors_t[:], ident16[:])
                    nc.scalar.copy(b1T[D_OBJ : 2 * D_OBJ, t * P : (t + 1) * P], tp2[:])

                # relation MLP, feature-major [features, EG]
                h1p = psp.tile([H_REL, EG], F32, tag="ps")
                nc.tensor.matmul(h1p[:], w1ab[:], b1T[:], start=True, stop=False)
                nc.tensor.matmul(
                    h1p[:], w1c[:], raT[:, g * EG : (g + 1) * EG],
                    start=False, stop=True,
                )
                h1T = sp.tile([H_REL, EG], F16, tag="hT")
                nc.scalar.activation(h1T[:], h1p[:], AF.Relu, bias=b1t[:])

                h2p = psp.tile([H_REL, EG], F32, tag="ps")
                nc.tensor.matmul(h2p[:], w2[:], h1T[:], start=True, stop=True)
                h2T = sp.tile([H_REL, EG], F16, tag="hT")
                nc.scalar.activation(h2T[:], h2p[:], AF.Relu, bias=b2t[:])

                h3p = psp.tile([H_REL, EG], F32, tag="ps")
                nc.tensor.matmul(h3p[:], w3[:], h2T[:], start=True, stop=True)
                h3T = sp.tile([H_REL, EG], F16, tag="hT")
                nc.scalar.activation(h3T[:], h3p[:], AF.Relu, bias=b3t[:])

                h4p = psp.tile([D_EFF, EG], F32, tag="ps")
                nc.tensor.matmul(h4p[:], w4[:], h3T[:], start=True, stop=True)
                eT = sp.tile([D_EFF, EG], F16, tag="eT")
                nc.scalar.activation(eT[:], h4p[:], AF.Relu, bias=b4t[:])

                # aggregate: e_agg.T += e_chunk.T @ one_hot(idx_r)_chunk
                for t in range(T):
                    ep = psp.tile([P, D_EFF], F16, tag="ps")
                    nc.tensor.transpose(
                        ep[:], eT[:, t * P : (t + 1) * P], ident16[:D_EFF, :D_EFF]
                    )
                    ec = ecp.tile([P, D_EFF], F16, tag="ec")
                    nc.scalar.copy(ec[:], ep[:])
                    first = g == 0 and t == 0
                    last = g == n_groups - 1 and t == T - 1
                    for q in range(n_obj // NQ):
                        nc.tensor.matmul(
                            agg_ps[:, q * NQ : (q + 1) * NQ],
                            ec[:],
                            rrt[t][:, q * NQ : (q + 1) * NQ],
                            start=first,
                            stop=last,
                        )

            # ---- all-reduce e_agg across cores -----------------------------
            eagg_sb = const.tile([D_EFF, n_obj], F32)
            nc.scalar.copy(eagg_sb[:], agg_ps[:])
            cc_in = dp.tile([D_EFF, n_obj], F32)
            cc_out = dp.tile([D_EFF, n_obj], F32)
            nc.sync.dma_start(cc_in[:], eagg_sb[:])
            if use_collective:
                nc.gpsimd.collective_compute(
                    "AllReduce",
                    ALU.add,
                    replica_groups=[list(range(n_cores))],
                    ins=[cc_in.opt()],
                    outs=[cc_out.opt()],
                )
            else:
                nc.sync.dma_start(cc_out[:], cc_in[:])
            eaggT = const.tile([D_EFF, n_obj], F32)
            nc.sync.dma_start(eaggT[:], cc_out[:])
            eaggT16 = const.tile([D_EFF, n_obj], F16)
            nc.vector.tensor_copy(eaggT16[:], eaggT[:])

            # ---- node phase (object MLP) -----------------------------------
            pTt = const.tile([D_OUT, n_obj], F16)
            for q in range(n_nq):
                sl = slice(q * NQ, (q + 1) * NQ)
                cp = psp.tile([H_OBJ, NQ], F32, tag="ps")
                nc.tensor.matmul(cp[:], ow1a[:], objT[:, sl], start=True, stop=False)
                nc.tensor.matmul(cp[:], ow1b[:], eaggT16[:, sl], start=False, stop=True)
                hT = sp.tile([H_OBJ, NQ], F16, tag="hT")
                nc.scalar.activation(hT[:], cp[:], AF.Relu, bias=ob1t[:])
                pp = psp.tile([D_OUT, NQ], F32, tag="ps")
                nc.tensor.matmul(pp[:], ow2[:], hT[:], start=True, stop=True)
                nc.scalar.activation(pTt[:, sl], pp[:], AF.Identity, bias=ob2t[:])
            nc.sync.dma_start(pT_d[:, :], pTt[:])

    nc.compile()
    return nc


class _Res:
    """Minimal stand-in for BassKernelResults (no trace support)."""

    def __init__(self, results):
        self.results = results
        self.exec_time_ns = None
        self.mean_exec_time_ns = None
        self.instructions_and_trace = None
        self.profile_json = None


def _make_runner(nc, n_cores):
    """Build the jitted shard_map executable ONCE; warm calls only pay
    input upload + execution (run_bass_via_pjrt re-creates the closure and
    re-traces on every call)."""
    import jax
    from jax.experimental.shard_map import shard_map
    from jax.sharding import Mesh, PartitionSpec

    from concourse.bass2jax import (
        _bass_exec_p,
        install_neuronx_cc_hook,
        partition_id_tensor,
    )

    install_neuronx_cc_hook()

    partition_name = nc.partition_id_tensor.name if nc.partition_id_tensor else None
    dbg_name = nc.dbg_addr.name if nc.dbg_addr is not None else None

    in_names = []
    out_names = []
    out_avals = []
    out_shapes = []
    for alloc in nc.m.functions[0].allocations:
        if not isinstance(alloc, mybir.MemoryLocationSet):
            continue
        name = alloc.memorylocations[0].name
        if alloc.kind == "ExternalInput":
            if name != partition_name:
                in_names.append(name)
        elif alloc.kind == "ExternalOutput":
            shape = tuple(alloc.tensor_shape)
            dtype = mybir.dt.np(alloc.dtype)
            out_names.append(name)
            out_avals.append(jax.core.ShapedArray(shape, dtype))
            out_shapes.append((shape, dtype))
    n_params = len(in_names)
    all_names = list(in_names) + list(out_names)
    if partition_name is not None:
        all_names.append(partition_name)

    donate = tuple(range(n_params, n_params + len(out_names)))

    def _body(*args):
        operands = list(args)
        if partition_name is not None:
            operands.append(partition_id_tensor())
        outs = _bass_exec_p.bind(
            *operands,
            out_avals=tuple(out_avals),
            in_names=tuple(all_names),
            out_names=tuple(out_names),
            lowering_input_output_aliases=(),
            sim_require_finite=True,
            sim_require_nnan=True,
            nc=nc,
        )
        return tuple(outs)

    devices = jax.devices()[:n_cores]
    assert len(devices) == n_cores
    mesh = Mesh(np.asarray(devices), ("core",))
    replicated = set()  # all inputs are per-core shards
    in_specs = tuple(
        PartitionSpec() if nm in replicated else PartitionSpec("core")
        for nm in in_names
    ) + (PartitionSpec("core"),) * len(out_names)
    out_specs = (PartitionSpec("core"),) * len(out_names)
    sharded = jax.jit(
        shard_map(
            _body, mesh=mesh, in_specs=in_specs, out_specs=out_specs,
            check_rep=False,
        ),
        donate_argnums=donate,
        keep_unused=True,
    )

    zeros_cache = [
        np.zeros((n_cores * shape[0], *shape[1:]), dtype)
        for shape, dtype in out_shapes
    ]
    dbg_zero = (
        np.zeros((n_cores, 2), np.uint32) if dbg_name is not None else None
    )

    def run(named_flats):
        """named_flats: dict input-name -> already-concatenated global array
        ([n_cores * per_core_len, ...])."""
        if dbg_name is not None:
            named_flats = {**named_flats, dbg_name: dbg_zero}
        concat_in = [named_flats[nm] for nm in in_names]
        out_arrs = sharded(*concat_in, *zeros_cache)
        # fetch only core 0's shard (all cores produce the full output)
        results0 = {}
        for i, name in enumerate(out_names):
            arr = out_arrs[i]
            try:
                shard0 = np.asarray(arr.addressable_shards[0].data)
                if shard0.shape != out_shapes[i][0]:
                    shard0 = shard0.reshape(n_cores, *out_shapes[i][0])[0]
            except Exception:
                shard0 = np.asarray(arr).reshape(n_cores, *out_shapes[i][0])[0]
            results0[name] = shard0
        return [results0]

    return run


_CACHE = {}
TRACE = False  # kept for test.py compat; tracing unsupported on this setup

_SCAN_SRC = r"""
#include <string.h>
/* One-hot rows contain exactly one 1.0f (bytes 00 00 80 3F) among 0.0f
   (all-zero bytes), so the first 0x3F byte sits at byte 4*idx+3. glibc's
   SIMD memchr with per-row early exit beats a full BLAS sgemv read. */
void onehot_memchr(const float* af, long n, long m, float* out) {
    const char* a = (const char*)af;
    long rowb = m * 4;
    for (long i = 0; i < n; i++) {
        const char* row = a + i * rowb;
        const char* p = (const char*)memchr(row, 0x3F, rowb);
        out[i] = p ? (float)((p - row - 3) >> 2) : 0.0f;
    }
}
"""


def _get_scanner():
    """Returns scan(a)->float32 indices for one-hot rows; C memchr fast
    path with a BLAS sgemv fallback."""
    if "scan" in _CACHE:
        return _CACHE["scan"]
    scan = None
    try:
        import importlib
        import tempfile

        import cffi

        ffi = cffi.FFI()
        ffi.cdef("void onehot_memchr(const float* a, long n, long m, float* out);")
        d = tempfile.mkdtemp()
        ffi.set_source("_onehot_scan_knl", _SCAN_SRC,
                       extra_compile_args=["-O3"])
        ffi.compile(tmpdir=d)
        if d not in sys.path:
            sys.path.insert(0, d)
        mod = importlib.import_module("_onehot_scan_knl")

        def scan(a):
            a = np.ascontiguousarray(a, dtype=np.float32)
            out = np.empty(a.shape[0], np.float32)
            mod.lib.onehot_memchr(
                mod.ffi.cast("float*", a.ctypes.data),
                a.shape[0], a.shape[1],
                mod.ffi.cast("float*", out.ctypes.data),
            )
            return out

        # self-check on a tiny case before trusting it
        chk = np.zeros((4, 8), np.float32)
        chk[[0, 1, 2, 3], [5, 0, 7, 3]] = 1.0
        assert np.array_equal(scan(chk), np.array([5, 0, 7, 3], np.float32))
    except Exception as e:
        print(f"kernel: memchr scanner unavailable ({e!r}); using BLAS",
              file=sys.stderr)

        def scan(a):
            a = np.asarray(a, dtype=np.float32)
            return a @ np.arange(a.shape[1], dtype=np.float32)

    _CACHE["scan"] = scan
    return scan


def _get_nc():
    if "nc" not in _CACHE:
        _CACHE["nc"] = build()
    return _CACHE["nc"]


def _pack_inputs(inputs):
    """Host-side marshalling: exact index extraction + packed f16 shards
    (preallocated buffers reused across calls)."""
    f32 = lambda k: np.asarray(inputs[k], dtype=np.float32)
    rr, rs, ra = f32("rr"), f32("rs"), f32("ra")
    obj = f32("obj")
    scan = _get_scanner()
    idx_r = scan(rr)  # exact: single 1.0 per row, values < 2^11
    idx_s = scan(rs)

    if "bufs" not in _CACHE:
        _CACHE["bufs"] = np.zeros((N_CORES, SSZ), np.float16)
    shard = _CACHE["bufs"]

    # int8-quantize ra per feature; fold the dequant scale into w1c so the
    # device MLP is unchanged: ra @ w1c == q @ (diag(s) @ w1c)
    ra_s = np.maximum(np.abs(ra).max(axis=0) / 127.0, 1e-12).astype(np.float32)
    ra_q = np.rint(ra / ra_s).astype(np.int8)
    w1c_scaled = f32("rm_w1")[P : P + D_REL] * ra_s[:, None]

    w_flat = np.zeros(W_PAD, np.float16)
    pieces = [
        f32("rm_w1")[0:P], w1c_scaled, f32("rm_b1"),
        f32("rm_w2"), f32("rm_b2"), f32("rm_w3"), f32("rm_b3"),
        f32("rm_w4"), f32("rm_b4"),
        f32("om_w1")[0:D_OBJ], f32("om_w1")[D_OBJ : D_OBJ + D_EFF],
        f32("om_b1"), f32("om_w2"), f32("om_b2"),
    ]
    o = 0
    for p_ in pieces:
        n = p_.size
        w_flat[o : o + n] = p_.astype(np.float16).ravel()
        o += n
    assert o == W_TOTAL

    obj16 = obj.astype(np.float16).ravel()
    for c in range(N_CORES):
        shard[c, 0:OBJ_SH] = obj16[c * OBJ_SH : (c + 1) * OBJ_SH]
        shard[c, OBJ_SH : OBJ_SH + W_SH] = w_flat[c * W_SH : (c + 1) * W_SH]

    idx_r3 = idx_r.reshape(N_CORES, NCH, P)
    idx_s3 = idx_s.reshape(N_CORES, NCH, P)
    for c in range(N_CORES):
        shard[c, O_IR : O_IR + EPC] = idx_r3[c].T.astype(np.float16).ravel()
        shard[c, O_IS : O_IS + EPC] = idx_s3[c].T.astype(np.float16).ravel()
        shard[c, O_RA : O_RA + RA_SLOTS].view(np.int8)[...] = (
            ra_q[c * EPC : (c + 1) * EPC].T.ravel()
        )
    return shard


def kernel(**inputs):
    nc = _get_nc()
    shard = _pack_inputs(inputs)

    if "runner" not in _CACHE:
        try:
            _CACHE["runner"] = _make_runner(nc, N_CORES)
        except Exception as e:
            print(f"kernel: cached runner unavailable ({e!r}); "
                  f"falling back to run_bass_kernel_spmd", file=sys.stderr)
            _CACHE["runner"] = None
    runner = _CACHE["runner"]
    if runner is not None:
        results = runner({"shard": shard.reshape(-1)})
        res = _Res(results)
    else:
        in_maps = [{"shard": shard[c]} for c in range(N_CORES)]
        res = run_bass_kernel_spmd(
            nc, in_maps, core_ids=list(range(N_CORES)), trace=False
        )
    _CACHE["last_results"] = res
    return np.ascontiguousarray(res.results[0]["pT"].T.astype(np.float32))


# revision 67
# speedup vs baseline: 1.4205x; 1.0139x over previous
"""InteractionNetwork (GNN message passing) Bass kernel for 8 Trainium2 cores.

Strategy (edge-sharded, per sharding hint):
  - The rr/rs inputs are one-hot by construction, so the host extracts the
    receiver/sender indices exactly (one sgemv with an arange vector each)
    and ships ONE packed f16 buffer per core (~330KB: a 1/8 shard of
    obj+weights, edge indices, and the ra.T slice) instead of the 512MB
    dense one-hot matrices — ~2.7MB total on the wire per call.
  - On device, obj and the MLP weights are reassembled from the shards with
    two AllGathers (staged through SBUF into DRAM pool tiles, since
    collectives cannot read IO tensors), so the host uploads them once.
  - Each core handles 4096 edges: node features are gathered with indirect
    DMA by index; the 4-layer relation MLP runs feature-major on the PE in
    f16 (f32 PSUM); for the rr.T @ e aggregation the one-hot receiver rows
    are rebuilt on-device (is_equal against an iota) and used as the moving
    operand of an accumulating matmul into a pinned PSUM e_agg.T
    accumulator.
  - Partial e_agg is AllReduce-summed in f32 across the 8 cores; every core
    runs the small object MLP on all 2048 nodes; host fetches core 0's
    output shard only.
  - The jitted shard_map executable is built once and cached, so warm calls
    only pay input upload + execution.
"""

import os
import sys

import numpy as np

os.environ.setdefault("MYCRO_LOCAL_CACHE", "1")
for _p in ("/opt/trn_rl_repo",):
    if os.path.isdir(_p) and _p not in sys.path:
        sys.path.insert(0, _p)

import concourse.bacc as bacc
import concourse.bass as bass
import concourse.mybir as mybir
import concourse.tile as tile
from concourse.bass_utils import run_bass_kernel_spmd
from concourse.masks import make_identity

P = 128
F32 = mybir.dt.float32
F16 = mybir.dt.float16
I32 = mybir.dt.int32
I16 = mybir.dt.int16
I8 = mybir.dt.int8
AF = mybir.ActivationFunctionType
ALU = mybir.AluOpType

N_OBJ, N_REL = 2048, 32768
D_OBJ, D_REL, D_EFF = 64, 32, 64
H_REL, H_OBJ = 128, 128
D_OUT = 3
N_CORES = 8

EPC = N_REL // N_CORES        # 4096 edges per core
NCH = EPC // P                # 32 chunks of 128 edges
OBJ_SH_R = N_OBJ // N_CORES   # 256 obj rows per core

# ---- packed weight blob layout (f16 elements) -----------------------------
_W_PIECES = [
    # (name, rows, cols) in packing order; loaded as [rows, cols]
    ("w1ab", P, H_REL),          # rm_w1[0:128]
    ("w1c", D_REL, H_REL),       # rm_w1[128:160]
    ("b1", H_REL, 1),
    ("w2", H_REL, H_REL),
    ("b2", H_REL, 1),
    ("w3", H_REL, H_REL),
    ("b3", H_REL, 1),
    ("w4", H_REL, D_EFF),
    ("b4", D_EFF, 1),
    ("ow1a", D_OBJ, H_OBJ),      # om_w1[0:64]
    ("ow1b", D_EFF, H_OBJ),      # om_w1[64:128]
    ("ob1", H_OBJ, 1),
    ("ow2", H_OBJ, D_OUT),
    ("ob2", D_OUT, 1),
]
_W_OFF = {}
_off = 0
for _nm, _r, _c in _W_PIECES:
    _W_OFF[_nm] = _off
    _off += _r * _c
W_TOTAL = _off                                  # 78787
W_PAD = 78848                                   # aligned pad

# ---- input blob layouts (f16 elements) ------------------------------------
# cshard: per-core 1/8 shard of (obj | weights); the device AllGathers the
#   full copies so the host uploads obj+weights once, not 8x.
# eblob: per-core edge shard: receiver/sender indices + ra.T slice
OBJ_SH = N_OBJ * D_OBJ // N_CORES               # 16384
W_SH = W_PAD // N_CORES                         # 9856
CSH = OBJ_SH + W_SH                             # per-core const-shard elems
O_IR = CSH
O_IS = O_IR + EPC
O_RA = O_IS + EPC                               # ra.T as int8 (2 per f16 slot)
RA_SLOTS = D_REL * EPC // 2                     # 65536 f16 slots
SSZ = O_RA + RA_SLOTS                           # one packed shard per core


def build(n_cores=N_CORES, use_collective=True):
    EG = 512                  # edges per MLP group
    T = EG // P               # 128-edge chunks per group
    n_groups = EPC // EG
    NQ = 512                  # node chunk (psum bank) for wide matmuls
    n_nq = N_OBJ // NQ
    n_obj = N_OBJ

    nc = bacc.Bacc(
        "TRN2",
        target_bir_lowering=False,
        debug=False,
        enable_asserts=False,
        num_devices=n_cores,
    )

    shard = nc.dram_tensor("shard", [SSZ], F16, kind="ExternalInput")
    pT_d = nc.dram_tensor("pT", [D_OUT, n_obj], F16, kind="ExternalOutput")

    with tile.TileContext(nc) as tc:
        with (
            tc.tile_pool(name="const", bufs=1) as const,
            tc.tile_pool(name="stream", bufs=8) as sp,
            tc.tile_pool(name="gat", bufs=4) as gp,
            tc.tile_pool(name="ec", bufs=8) as ecp,
            tc.tile_pool(name="aggp", bufs=1, space="PSUM") as aggp,
            tc.tile_pool(name="psp", bufs=4, space="PSUM") as psp,
            tc.tile_pool(name="dram", bufs=1, space="DRAM") as dp,
        ):
            # assemble full obj + weights from the per-core shards.
            # collectives cannot read IO tensors, so bounce the shard
            # through SBUF into a DRAM pool tile first.
            CCOL = CSH // P
            objall_d = dp.tile([n_obj * D_OBJ], F16)
            wall_d = dp.tile([W_PAD], F16)
            cstage = dp.tile([CSH], F16)
            with tc.tile_pool(name="stage", bufs=1) as stp:
                cs_sb = stp.tile([P, CCOL], F16)
                nc.sync.dma_start(
                    cs_sb[:], shard[0:CSH].rearrange("(p c) -> p c", c=CCOL)
                )
                nc.sync.dma_start(
                    cstage[0:CSH].rearrange("(p c) -> p c", c=CCOL), cs_sb[:]
                )
            if use_collective:
                nc.gpsimd.collective_compute(
                    "AllGather",
                    ALU.bypass,
                    replica_groups=[list(range(n_cores))],
                    ins=[cstage[0:OBJ_SH].opt()],
                    outs=[objall_d.opt()],
                )
                nc.gpsimd.collective_compute(
                    "AllGather",
                    ALU.bypass,
                    replica_groups=[list(range(n_cores))],
                    ins=[cstage[OBJ_SH : OBJ_SH + W_SH].opt()],
                    outs=[wall_d.opt()],
                )
            else:
                nc.sync.dma_start(objall_d[0:OBJ_SH], cstage[0:OBJ_SH])
                nc.sync.dma_start(
                    wall_d[0:W_SH], cstage[OBJ_SH : OBJ_SH + W_SH]
                )
            obj2d = objall_d[:].rearrange("(n d) -> n d", d=D_OBJ)

            # ---- constants -------------------------------------------------
            ident32 = const.tile([P, P], F32)
            make_identity(nc, ident32[:])
            ident16 = const.tile([P, P], F16)
            make_identity(nc, ident16[:])

            iota_i = const.tile([P, n_obj], I16)
            nc.gpsimd.iota(iota_i[:], pattern=[[1, n_obj]], base=0, channel_multiplier=0)
            iota16 = const.tile([P, n_obj], F16)
            nc.vector.tensor_copy(iota16[:], iota_i[:])

            def wmat(nm, r, c):
                # NB: explicit per-weight tag — a shared tag would make all
                # weight tiles rotate through one bufs=1 slot and deadlock
                # (slot release waits on the last MLP group).
                t = const.tile([r, c], F16, tag=f"w_{nm}")
                o = _W_OFF[nm]
                nc.sync.dma_start(
                    t[:], wall_d[o : o + r * c].rearrange("(k m) -> k m", m=c)
                )
                return t

            def wcol(nm, r):
                th = const.tile([r, 1], F16, tag=f"bh_{nm}")
                o = _W_OFF[nm]
                nc.sync.dma_start(
                    th[:], wall_d[o : o + r].rearrange("(k m) -> k m", m=1)
                )
                t = const.tile([r, 1], F32, tag=f"b_{nm}")
                nc.vector.tensor_copy(t[:], th[:])
                return t

            w1ab = wmat("w1ab", P, H_REL)
            w1c = wmat("w1c", D_REL, H_REL)
            w2 = wmat("w2", H_REL, H_REL)
            w3 = wmat("w3", H_REL, H_REL)
            w4 = wmat("w4", H_REL, D_EFF)
            ow1a = wmat("ow1a", D_OBJ, H_OBJ)
            ow1b = wmat("ow1b", D_EFF, H_OBJ)
            ow2 = wmat("ow2", H_OBJ, D_OUT)
            b1t = wcol("b1", H_REL)
            b2t = wcol("b2", H_REL)
            b3t = wcol("b3", H_REL)
            b4t = wcol("b4", D_EFF)
            ob1t = wcol("ob1", H_OBJ)
            ob2t = wcol("ob2", D_OUT)

            # edge indices: f16 (one-hot rebuild), f32 -> i32 (indirect DMA)
            idxr_h = const.tile([P, NCH], F16)
            nc.sync.dma_start(
                idxr_h[:], shard[O_IR : O_IR + EPC].rearrange("(p c) -> p c", c=NCH)
            )
            idxs_h = const.tile([P, NCH], F16)
            nc.sync.dma_start(
                idxs_h[:], shard[O_IS : O_IS + EPC].rearrange("(p c) -> p c", c=NCH)
            )
            idxr_f = const.tile([P, NCH], F32)
            nc.vector.tensor_copy(idxr_f[:], idxr_h[:])
            idxs_f = const.tile([P, NCH], F32)
            nc.vector.tensor_copy(idxs_f[:], idxs_h[:])
            idxr_i = const.tile([P, NCH], I32)
            nc.vector.tensor_copy(idxr_i[:], idxr_f[:])
            idxs_i = const.tile([P, NCH], I32)
            nc.vector.tensor_copy(idxs_i[:], idxs_f[:])

            # ra.T arrives int8 (dequant scale folded into w1c on the host)
            ra_i8 = const.tile([D_REL, EPC], I8)
            shard_i8 = shard.bitcast(I8)
            nc.sync.dma_start(
                ra_i8[:],
                shard_i8[2 * O_RA : 2 * O_RA + D_REL * EPC].rearrange(
                    "(d e) -> d e", e=EPC
                ),
            )
            raT = const.tile([D_REL, EPC], F16)
            nc.vector.tensor_copy(raT[:], ra_i8[:])

            # obj.T in SBUF (for the node-model MLP)
            objT = const.tile([D_OBJ, n_obj], F16)
            for k in range(n_obj // P):
                ot = gp.tile([P, D_OBJ], F16, tag="objload")
                nc.sync.dma_start(ot[:], obj2d[k * P : (k + 1) * P, :])
                tp = psp.tile([D_OBJ, P], F16, tag="ps")
                nc.tensor.transpose(tp[:], ot[:], ident16[:])
                nc.scalar.copy(objT[:, k * P : (k + 1) * P], tp[:])

            # pinned accumulator: e_agg.T [64, n_obj] (4 PSUM banks)
            agg_ps = aggp.tile([D_EFF, n_obj], F32)

            # ---- edge phase ------------------------------------------------
            for g in range(n_groups):
                rrt = []
                b1T = sp.tile([P, EG], F16, tag="b1T")
                for t in range(T):
                    c = g * T + t
                    # one-hot receiver rows for the aggregation matmul
                    oh = sp.tile([P, n_obj], F16, tag="oh")
                    nc.vector.tensor_tensor(
                        out=oh[:],
                        in0=idxr_h[:, c : c + 1].to_broadcast([P, n_obj]),
                        in1=iota16[:],
                        op=ALU.is_equal,
                    )
                    rrt.append(oh)

                    orr_t = gp.tile([P, D_OBJ], F16, tag="gat")
                    nc.gpsimd.indirect_dma_start(
                        out=orr_t[:], out_offset=None, in_=obj2d,
                        in_offset=bass.IndirectOffsetOnAxis(
                            ap=idxr_i[:, c : c + 1], axis=0
                        ),
                    )
                    tp = psp.tile([D_OBJ, P], F16, tag="ps")
                    nc.tensor.transpose(tp[:], orr_t[:], ident16[:])
                    nc.scalar.copy(b1T[0:D_OBJ, t * P : (t + 1) * P], tp[:])

                    ors_t = gp.tile([P, D_OBJ], F16, tag="gat")
                    nc.gpsimd.indirect_dma_start(
                        out=ors_t[:], out_offset=None, in_=obj2d,
                        in_offset=bass.IndirectOffsetOnAxis(
                            ap=idxs_i[:, c : c + 1], axis=0
                        ),
                    )
                    tp2 = psp.tile([D_OBJ, P], F16, tag="ps")
                    nc.tensor.transpose(tp2[:], ors_t[:], ident16[:])
                    nc.scalar.copy(b1T[D_OBJ : 2 * D_OBJ, t * P : (t + 1) * P], tp2[:])

                # relation MLP, feature-major [features, EG]
                h1p = psp.tile([H_REL, EG], F32, tag="ps")
                nc.tensor.matmul(h1p[:], w1ab[:], b1T[:], start=True, stop=False)
                nc.tensor.matmul(
                    h1p[:], w1c[:], raT[:, g * EG : (g + 1) * EG],
                    start=False, stop=True,
                )
                h1T = sp.tile([H_REL, EG], F16, tag="hT")
                nc.scalar.activation(h1T[:], h1p[:], AF.Relu, bias=b1t[:])

                h2p = psp.tile([H_REL, EG], F32, tag="ps")
                nc.tensor.matmul(h2p[:], w2[:], h1T[:], start=True, stop=True)
                h2T = sp.tile([H_REL, EG], F16, tag="hT")
                nc.scalar.activation(h2T[:], h2p[:], AF.Relu, bias=b2t[:])

                h3p = psp.tile([H_REL, EG], F32, tag="ps")
                nc.tensor.matmul(h3p[:], w3[:], h2T[:], start=True, stop=True)
                h3T = sp.tile([H_REL, EG], F16, tag="hT")
                nc.scalar.activation(h3T[:], h3p[:], AF.Relu, bias=b3t[:])

                h4p = psp.tile([D_EFF, EG], F32, tag="ps")
                nc.tensor.matmul(h4p[:], w4[:], h3T[:], start=True, stop=True)
                eT = sp.tile([D_EFF, EG], F16, tag="eT")
                nc.scalar.activation(eT[:], h4p[:], AF.Relu, bias=b4t[:])

                # aggregate: e_agg.T += e_chunk.T @ one_hot(idx_r)_chunk
                for t in range(T):
                    ep = psp.tile([P, D_EFF], F16, tag="ps")
                    nc.tensor.transpose(
                        ep[:], eT[:, t * P : (t + 1) * P], ident16[:D_EFF, :D_EFF]
                    )
                    ec = ecp.tile([P, D_EFF], F16, tag="ec")
                    nc.scalar.copy(ec[:], ep[:])
                    first = g == 0 and t == 0
                    last = g == n_groups - 1 and t == T - 1
                    for q in range(n_obj // NQ):
                        nc.tensor.matmul(
                            agg_ps[:, q * NQ : (q + 1) * NQ],
                            ec[:],
                            rrt[t][:, q * NQ : (q + 1) * NQ],
                            start=first,
                            stop=last,
                        )

            # ---- all-reduce e_agg across cores -----------------------------
            eagg_sb = const.tile([D_EFF, n_obj], F32)
            nc.scalar.copy(eagg_sb[:], agg_ps[:])
            cc_in = dp.tile([D_EFF, n_obj], F32)
            cc_out = dp.tile([D_EFF, n_obj], F32)
            nc.sync.dma_start(cc_in[:], eagg_sb[:])
            if use_collective:
                nc.gpsimd.collective_compute(
                    "AllReduce",
                    ALU.add,
                    replica_groups=[list(range(n_cores))],
                    ins=[cc_in.opt()],
                    outs=[cc_out.opt()],
                )
            else:
                nc.sync.dma_start(cc_out[:], cc_in[:])
            eaggT = const.tile([D_EFF, n_obj], F32)
            nc.sync.dma_start(eaggT[:], cc_out[:])
            eaggT16 = const.tile([D_EFF, n_obj], F16)
            nc.vector.tensor_copy(eaggT16[:], eaggT[:])

            # ---- node phase (object MLP) -----------------------------------
            pTt = const.tile([D_OUT, n_obj], F16)
            for q in range(n_nq):
                sl = slice(q * NQ, (q + 1) * NQ)
                cp = psp.tile([H_OBJ, NQ], F32, tag="ps")
                nc.tensor.matmul(cp[:], ow1a[:], objT[:, sl], start=True, stop=False)
                nc.tensor.matmul(cp[:], ow1b[:], eaggT16[:, sl], start=False, stop=True)
                hT = sp.tile([H_OBJ, NQ], F16, tag="hT")
                nc.scalar.activation(hT[:], cp[:], AF.Relu, bias=ob1t[:])
                pp = psp.tile([D_OUT, NQ], F32, tag="ps")
                nc.tensor.matmul(pp[:], ow2[:], hT[:], start=True, stop=True)
                nc.scalar.activation(pTt[:, sl], pp[:], AF.Identity, bias=ob2t[:])
            nc.sync.dma_start(pT_d[:, :], pTt[:])

    nc.compile()
    return nc


class _Res:
    """Minimal stand-in for BassKernelResults (no trace support)."""

    def __init__(self, results):
        self.results = results
        self.exec_time_ns = None
        self.mean_exec_time_ns = None
        self.instructions_and_trace = None
        self.profile_json = None


def _make_runner(nc, n_cores):
    """Build the jitted shard_map executable ONCE; warm calls only pay
    input upload + execution (run_bass_via_pjrt re-creates the closure and
    re-traces on every call)."""
    import jax
    from jax.experimental.shard_map import shard_map
    from jax.sharding import Mesh, PartitionSpec

    from concourse.bass2jax import (
        _bass_exec_p,
        install_neuronx_cc_hook,
        partition_id_tensor,
    )

    install_neuronx_cc_hook()

    partition_name = nc.partition_id_tensor.name if nc.partition_id_tensor else None
    dbg_name = nc.dbg_addr.name if nc.dbg_addr is not None else None

    in_names = []
    out_names = []
    out_avals = []
    out_shapes = []
    for alloc in nc.m.functions[0].allocations:
        if not isinstance(alloc, mybir.MemoryLocationSet):
            continue
        name = alloc.memorylocations[0].name
        if alloc.kind == "ExternalInput":
            if name != partition_name:
                in_names.append(name)
        elif alloc.kind == "ExternalOutput":
            shape = tuple(alloc.tensor_shape)
            dtype = mybir.dt.np(alloc.dtype)
            out_names.append(name)
            out_avals.append(jax.core.ShapedArray(shape, dtype))
            out_shapes.append((shape, dtype))
    n_params = len(in_names)
    all_names = list(in_names) + list(out_names)
    if partition_name is not None:
        all_names.append(partition_name)

    donate = tuple(range(n_params, n_params + len(out_names)))

    def _body(*args):
        operands = list(args)
        if partition_name is not None:
            operands.append(partition_id_tensor())
        outs = _bass_exec_p.bind(
            *operands,
            out_avals=tuple(out_avals),
            in_names=tuple(all_names),
            out_names=tuple(out_names),
            lowering_input_output_aliases=(),
            sim_require_finite=True,
            sim_require_nnan=True,
            nc=nc,
        )
        return tuple(outs)

    devices = jax.devices()[:n_cores]
    assert len(devices) == n_cores
    mesh = Mesh(np.asarray(devices), ("core",))
    replicated = set()  # all inputs are per-core shards
    in_specs = tuple(
        PartitionSpec() if nm in replicated else PartitionSpec("core")
        for nm in in_names
    ) + (PartitionSpec("core"),) * len(out_names)
    out_specs = (PartitionSpec("core"),) * len(out_names)
    sharded = jax.jit(
        shard_map(
            _body, mesh=mesh, in_specs=in_specs, out_specs=out_specs,
            check_rep=False,
        ),
        donate_argnums=donate,
        keep_unused=True,
    )

    zeros_cache = [
        np.zeros((n_cores * shape[0], *shape[1:]), dtype)
        for shape, dtype in out_shapes
    ]
    dbg_zero = (
        np.zeros((n_cores, 2), np.uint32) if dbg_name is not None else None
    )

    def run(named_flats):
        """named_flats: dict input-name -> already-concatenated global array
        ([n_cores * per_core_len, ...])."""
        if dbg_name is not None:
            named_flats = {**named_flats, dbg_name: dbg_zero}
        concat_in = [named_flats[nm] for nm in in_names]
        out_arrs = sharded(*concat_in, *zeros_cache)
        # fetch only core 0's shard (all cores produce the full output)
        results0 = {}
        for i, name in enumerate(out_names):
            arr = out_arrs[i]
            try:
                shard0 = np.asarray(arr.addressable_shards[0].data)
                if shard0.shape != out_shapes[i][0]:
                    shard0 = shard0.reshape(n_cores, *out_shapes[i][0])[0]
            except Exception:
                shard0 = np.asarray(arr).reshape(n_cores, *out_shapes[i][0])[0]
            results0[name] = shard0
        return [results0]

    return run


_CACHE = {}
TRACE = False  # kept for test.py compat; tracing unsupported on this setup

_SCAN_SRC = r"""
#include <string.h>
#include <math.h>
/* One-hot rows contain exactly one 1.0f (bytes 00 00 80 3F) among 0.0f
   (all-zero bytes), so the first 0x3F byte sits at byte 4*idx+3. glibc's
   SIMD memchr with per-row early exit beats a full BLAS sgemv read. */
void onehot_memchr(const float* af, long n, long m, float* out) {
    const char* a = (const char*)af;
    long rowb = m * 4;
    for (long i = 0; i < n; i++) {
        const char* row = a + i * rowb;
        const char* p = (const char*)memchr(row, 0x3F, rowb);
        out[i] = p ? (float)((p - row - 3) >> 2) : 0.0f;
    }
}
/* One-pass int8 quantize + transpose + scatter of ra into the per-core
   blob regions: out[c][d*epc + e] = rint(ra[c*epc+e, d] * inv_s[d]).
   Blocked over 64 edges so reads stay in L1 and writes are 64B runs. */
void quant_ra(const float* ra, long n_cores, long epc, long d,
              const float* inv_s, char* out0, long core_stride) {
    for (long c = 0; c < n_cores; c++) {
        const float* rc = ra + c * epc * d;
        char* oc = out0 + c * core_stride;
        for (long e0 = 0; e0 < epc; e0 += 64) {
            for (long dd = 0; dd < d; dd++) {
                float s = inv_s[dd];
                char* op = oc + dd * epc + e0;
                const float* ip = rc + e0 * d + dd;
                for (long k = 0; k < 64; k++) {
                    op[k] = (char)lrintf(ip[k * d] * s);
                }
            }
        }
    }
}
"""


def _get_scanner():
    """Returns scan(a)->float32 indices for one-hot rows; C memchr fast
    path with a BLAS sgemv fallback."""
    if "scan" in _CACHE:
        return _CACHE["scan"]
    scan = None
    try:
        import importlib
        import tempfile

        import cffi

        ffi = cffi.FFI()
        ffi.cdef(
            "void onehot_memchr(const float* a, long n, long m, float* out);"
            "void quant_ra(const float* ra, long n_cores, long epc, long d,"
            "              const float* inv_s, char* out0, long core_stride);"
        )
        d = tempfile.mkdtemp()
        ffi.set_source("_onehot_scan_knl", _SCAN_SRC,
                       extra_compile_args=["-O3", "-march=native"])
        ffi.compile(tmpdir=d)
        if d not in sys.path:
            sys.path.insert(0, d)
        mod = importlib.import_module("_onehot_scan_knl")
        _CACHE["cmod"] = mod

        def scan(a):
            a = np.ascontiguousarray(a, dtype=np.float32)
            out = np.empty(a.shape[0], np.float32)
            mod.lib.onehot_memchr(
                mod.ffi.cast("float*", a.ctypes.data),
                a.shape[0], a.shape[1],
                mod.ffi.cast("float*", out.ctypes.data),
            )
            return out

        # self-check on a tiny case before trusting it
        chk = np.zeros((4, 8), np.float32)
        chk[[0, 1, 2, 3], [5, 0, 7, 3]] = 1.0
        assert np.array_equal(scan(chk), np.array([5, 0, 7, 3], np.float32))
    except Exception as e:
        print(f"kernel: memchr scanner unavailable ({e!r}); using BLAS",
              file=sys.stderr)

        def scan(a):
            a = np.asarray(a, dtype=np.float32)
            return a @ np.arange(a.shape[1], dtype=np.float32)

    _CACHE["scan"] = scan
    return scan


def _get_nc():
    if "nc" not in _CACHE:
        _CACHE["nc"] = build()
    return _CACHE["nc"]


def _pack_inputs(inputs):
    """Host-side marshalling: exact index extraction + packed f16 shards
    (preallocated buffers reused across calls)."""
    f32 = lambda k: np.asarray(inputs[k], dtype=np.float32)
    rr, rs, ra = f32("rr"), f32("rs"), f32("ra")
    obj = f32("obj")
    scan = _get_scanner()
    idx_r = scan(rr)  # exact: single 1.0 per row, values < 2^11
    idx_s = scan(rs)

    if "bufs" not in _CACHE:
        _CACHE["bufs"] = np.zeros((N_CORES, SSZ), np.float16)
    shard = _CACHE["bufs"]

    # int8-quantize ra per feature; fold the dequant scale into w1c so the
    # device MLP is unchanged: ra @ w1c == q @ (diag(s) @ w1c)
    ra = np.ascontiguousarray(ra)
    ra_s = np.maximum(np.abs(ra).max(axis=0) / 127.0, 1e-12).astype(np.float32)
    w1c_scaled = f32("rm_w1")[P : P + D_REL] * ra_s[:, None]

    w_flat = np.zeros(W_PAD, np.float16)
    pieces = [
        f32("rm_w1")[0:P], w1c_scaled, f32("rm_b1"),
        f32("rm_w2"), f32("rm_b2"), f32("rm_w3"), f32("rm_b3"),
        f32("rm_w4"), f32("rm_b4"),
        f32("om_w1")[0:D_OBJ], f32("om_w1")[D_OBJ : D_OBJ + D_EFF],
        f32("om_b1"), f32("om_w2"), f32("om_b2"),
    ]
    o = 0
    for p_ in pieces:
        n = p_.size
        w_flat[o : o + n] = p_.astype(np.float16).ravel()
        o += n
    assert o == W_TOTAL

    obj16 = obj.astype(np.float16).ravel()
    for c in range(N_CORES):
        shard[c, 0:OBJ_SH] = obj16[c * OBJ_SH : (c + 1) * OBJ_SH]
        shard[c, OBJ_SH : OBJ_SH + W_SH] = w_flat[c * W_SH : (c + 1) * W_SH]

    idx_r3 = idx_r.reshape(N_CORES, NCH, P)
    idx_s3 = idx_s.reshape(N_CORES, NCH, P)
    for c in range(N_CORES):
        shard[c, O_IR : O_IR + EPC] = idx_r3[c].T.astype(np.float16).ravel()
        shard[c, O_IS : O_IS + EPC] = idx_s3[c].T.astype(np.float16).ravel()

    mod = _CACHE.get("cmod")
    if mod is not None:
        inv_s = np.ascontiguousarray(1.0 / ra_s, dtype=np.float32)
        mod.lib.quant_ra(
            mod.ffi.cast("float*", ra.ctypes.data),
            N_CORES, EPC, D_REL,
            mod.ffi.cast("float*", inv_s.ctypes.data),
            mod.ffi.cast("char*", shard.ctypes.data) + 2 * O_RA,
            2 * SSZ,
        )
    else:
        ra_q = np.rint(ra / ra_s).astype(np.int8)
        for c in range(N_CORES):
            shard[c, O_RA : O_RA + RA_SLOTS].view(np.int8)[...] = (
                ra_q[c * EPC : (c + 1) * EPC].T.ravel()
            )
    return shard


def kernel(**inputs):
    nc = _get_nc()
    shard = _pack_inputs(inputs)

    if "runner" not in _CACHE:
        try:
            _CACHE["runner"] = _make_runner(nc, N_CORES)
        except Exception as e:
            print(f"kernel: cached runner unavailable ({e!r}); "
                  f"falling back to run_bass_kernel_spmd", file=sys.stderr)
            _CACHE["runner"] = None
    runner = _CACHE["runner"]
    if runner is not None:
        results = runner({"shard": shard.reshape(-1)})
        res = _Res(results)
    else:
        in_maps = [{"shard": shard[c]} for c in range(N_CORES)]
        res = run_bass_kernel_spmd(
            nc, in_maps, core_ids=list(range(N_CORES)), trace=False
        )
    _CACHE["last_results"] = res
    return np.ascontiguousarray(res.results[0]["pT"].T.astype(np.float32))


# revision 68
# speedup vs baseline: 1.4588x; 1.0270x over previous
"""InteractionNetwork (GNN message passing) Bass kernel for 8 Trainium2 cores.

Strategy (edge-sharded, per sharding hint):
  - The rr/rs inputs are one-hot by construction, so the host extracts the
    receiver/sender indices exactly (one sgemv with an arange vector each)
    and ships ONE packed f16 buffer per core (~330KB: a 1/8 shard of
    obj+weights, edge indices, and the ra.T slice) instead of the 512MB
    dense one-hot matrices — ~2.7MB total on the wire per call.
  - On device, obj and the MLP weights are reassembled from the shards with
    two AllGathers (staged through SBUF into DRAM pool tiles, since
    collectives cannot read IO tensors), so the host uploads them once.
  - Each core handles 4096 edges: node features are gathered with indirect
    DMA by index; the 4-layer relation MLP runs feature-major on the PE in
    f16 (f32 PSUM); for the rr.T @ e aggregation the one-hot receiver rows
    are rebuilt on-device (is_equal against an iota) and used as the moving
    operand of an accumulating matmul into a pinned PSUM e_agg.T
    accumulator.
  - Partial e_agg is AllReduce-summed in f32 across the 8 cores; every core
    runs the small object MLP on all 2048 nodes; host fetches core 0's
    output shard only.
  - The jitted shard_map executable is built once and cached, so warm calls
    only pay input upload + execution.
"""

import os
import sys

import numpy as np

os.environ.setdefault("MYCRO_LOCAL_CACHE", "1")
for _p in ("/opt/trn_rl_repo",):
    if os.path.isdir(_p) and _p not in sys.path:
        sys.path.insert(0, _p)

import concourse.bacc as bacc
import concourse.bass as bass
import concourse.mybir as mybir
import concourse.tile as tile
from concourse.bass_utils import run_bass_kernel_spmd
from concourse.masks import make_identity

P = 128
F32 = mybir.dt.float32
F16 = mybir.dt.float16
I32 = mybir.dt.int32
I16 = mybir.dt.int16
I8 = mybir.dt.int8
AF = mybir.ActivationFunctionType
ALU = mybir.AluOpType

N_OBJ, N_REL = 2048, 32768
D_OBJ, D_REL, D_EFF = 64, 32, 64
H_REL, H_OBJ = 128, 128
D_OUT = 3
N_CORES = 8

EPC = N_REL // N_CORES        # 4096 edges per core
NCH = EPC // P                # 32 chunks of 128 edges
OBJ_SH_R = N_OBJ // N_CORES   # 256 obj rows per core

# ---- packed weight blob layout (f16 elements) -----------------------------
_W_PIECES = [
    # (name, rows, cols) in packing order; loaded as [rows, cols]
    ("w1ab", P, H_REL),          # rm_w1[0:128]
    ("w1c", D_REL, H_REL),       # rm_w1[128:160]
    ("b1", H_REL, 1),
    ("w2", H_REL, H_REL),
    ("b2", H_REL, 1),
    ("w3", H_REL, H_REL),
    ("b3", H_REL, 1),
    ("w4", H_REL, D_EFF),
    ("b4", D_EFF, 1),
    ("ow1a", D_OBJ, H_OBJ),      # om_w1[0:64]
    ("ow1b", D_EFF, H_OBJ),      # om_w1[64:128]
    ("ob1", H_OBJ, 1),
    ("ow2", H_OBJ, D_OUT),
    ("ob2", D_OUT, 1),
]
_W_OFF = {}
_off = 0
for _nm, _r, _c in _W_PIECES:
    _W_OFF[_nm] = _off
    _off += _r * _c
W_TOTAL = _off                                  # 78787
W_PAD = 78848                                   # aligned pad

# ---- input blob layouts (f16 elements) ------------------------------------
# cshard: per-core 1/8 shard of (obj | weights); the device AllGathers the
#   full copies so the host uploads obj+weights once, not 8x.
# eblob: per-core edge shard: receiver/sender indices + ra.T slice
OBJ_SH = N_OBJ * D_OBJ // N_CORES               # 16384
W_SH = W_PAD // N_CORES                         # 9856
CSH = OBJ_SH + W_SH                             # per-core const-shard elems
O_IR = CSH
O_IS = O_IR + EPC
O_RA = O_IS + EPC                               # ra.T as int8 (2 per f16 slot)
RA_SLOTS = D_REL * EPC // 2                     # 65536 f16 slots
SSZ = O_RA + RA_SLOTS                           # one packed shard per core


def build(n_cores=N_CORES, use_collective=True):
    EG = 512                  # edges per MLP group
    T = EG // P               # 128-edge chunks per group
    n_groups = EPC // EG
    NQ = 512                  # node chunk (psum bank) for wide matmuls
    n_nq = N_OBJ // NQ
    n_obj = N_OBJ

    nc = bacc.Bacc(
        "TRN2",
        target_bir_lowering=False,
        debug=False,
        enable_asserts=False,
        num_devices=n_cores,
    )

    shard = nc.dram_tensor("shard", [SSZ], F16, kind="ExternalInput")
    pT_d = nc.dram_tensor("pT", [D_OUT, n_obj], F16, kind="ExternalOutput")

    with tile.TileContext(nc) as tc:
        with (
            tc.tile_pool(name="const", bufs=1) as const,
            tc.tile_pool(name="stream", bufs=8) as sp,
            tc.tile_pool(name="gat", bufs=4) as gp,
            tc.tile_pool(name="ec", bufs=8) as ecp,
            tc.tile_pool(name="aggp", bufs=1, space="PSUM") as aggp,
            tc.tile_pool(name="psp", bufs=4, space="PSUM") as psp,
            tc.tile_pool(name="dram", bufs=1, space="DRAM") as dp,
        ):
            # assemble full obj + weights from the per-core shards.
            # collectives cannot read IO tensors, so bounce the shard
            # through SBUF into a DRAM pool tile first.
            CCOL = CSH // P
            objall_d = dp.tile([n_obj * D_OBJ], F16)
            wall_d = dp.tile([W_PAD], F16)
            cstage = dp.tile([CSH], F16)
            with tc.tile_pool(name="stage", bufs=1) as stp:
                cs_sb = stp.tile([P, CCOL], F16)
                nc.sync.dma_start(
                    cs_sb[:], shard[0:CSH].rearrange("(p c) -> p c", c=CCOL)
                )
                nc.sync.dma_start(
                    cstage[0:CSH].rearrange("(p c) -> p c", c=CCOL), cs_sb[:]
                )
            if use_collective:
                nc.gpsimd.collective_compute(
                    "AllGather",
                    ALU.bypass,
                    replica_groups=[list(range(n_cores))],
                    ins=[cstage[0:OBJ_SH].opt()],
                    outs=[objall_d.opt()],
                )
                nc.gpsimd.collective_compute(
                    "AllGather",
                    ALU.bypass,
                    replica_groups=[list(range(n_cores))],
                    ins=[cstage[OBJ_SH : OBJ_SH + W_SH].opt()],
                    outs=[wall_d.opt()],
                )
            else:
                nc.sync.dma_start(objall_d[0:OBJ_SH], cstage[0:OBJ_SH])
                nc.sync.dma_start(
                    wall_d[0:W_SH], cstage[OBJ_SH : OBJ_SH + W_SH]
                )
            obj2d = objall_d[:].rearrange("(n d) -> n d", d=D_OBJ)

            # ---- constants -------------------------------------------------
            ident32 = const.tile([P, P], F32)
            make_identity(nc, ident32[:])
            ident16 = const.tile([P, P], F16)
            make_identity(nc, ident16[:])

            iota_i = const.tile([P, n_obj], I16)
            nc.gpsimd.iota(iota_i[:], pattern=[[1, n_obj]], base=0, channel_multiplier=0)
            iota16 = const.tile([P, n_obj], F16)
            nc.vector.tensor_copy(iota16[:], iota_i[:])

            def wmat(nm, r, c):
                # NB: explicit per-weight tag — a shared tag would make all
                # weight tiles rotate through one bufs=1 slot and deadlock
                # (slot release waits on the last MLP group).
                t = const.tile([r, c], F16, tag=f"w_{nm}")
                o = _W_OFF[nm]
                nc.sync.dma_start(
                    t[:], wall_d[o : o + r * c].rearrange("(k m) -> k m", m=c)
                )
                return t

            def wcol(nm, r):
                th = const.tile([r, 1], F16, tag=f"bh_{nm}")
                o = _W_OFF[nm]
                nc.sync.dma_start(
                    th[:], wall_d[o : o + r].rearrange("(k m) -> k m", m=1)
                )
                t = const.tile([r, 1], F32, tag=f"b_{nm}")
                nc.vector.tensor_copy(t[:], th[:])
                return t

            w1ab = wmat("w1ab", P, H_REL)
            w1c = wmat("w1c", D_REL, H_REL)
            w2 = wmat("w2", H_REL, H_REL)
            w3 = wmat("w3", H_REL, H_REL)
            w4 = wmat("w4", H_REL, D_EFF)
            ow1a = wmat("ow1a", D_OBJ, H_OBJ)
            ow1b = wmat("ow1b", D_EFF, H_OBJ)
            ow2 = wmat("ow2", H_OBJ, D_OUT)
            b1t = wcol("b1", H_REL)
            b2t = wcol("b2", H_REL)
            b3t = wcol("b3", H_REL)
            b4t = wcol("b4", D_EFF)
            ob1t = wcol("ob1", H_OBJ)
            ob2t = wcol("ob2", D_OUT)

            # edge indices: f16 (one-hot rebuild), f32 -> i32 (indirect DMA)
            idxr_h = const.tile([P, NCH], F16)
            nc.sync.dma_start(
                idxr_h[:], shard[O_IR : O_IR + EPC].rearrange("(p c) -> p c", c=NCH)
            )
            idxs_h = const.tile([P, NCH], F16)
            nc.sync.dma_start(
                idxs_h[:], shard[O_IS : O_IS + EPC].rearrange("(p c) -> p c", c=NCH)
            )
            idxr_f = const.tile([P, NCH], F32)
            nc.vector.tensor_copy(idxr_f[:], idxr_h[:])
            idxs_f = const.tile([P, NCH], F32)
            nc.vector.tensor_copy(idxs_f[:], idxs_h[:])
            idxr_i = const.tile([P, NCH], I32)
            nc.vector.tensor_copy(idxr_i[:], idxr_f[:])
            idxs_i = const.tile([P, NCH], I32)
            nc.vector.tensor_copy(idxs_i[:], idxs_f[:])

            # ra.T arrives int8 (dequant scale folded into w1c on the host)
            ra_i8 = const.tile([D_REL, EPC], I8)
            shard_i8 = shard.bitcast(I8)
            nc.sync.dma_start(
                ra_i8[:],
                shard_i8[2 * O_RA : 2 * O_RA + D_REL * EPC].rearrange(
                    "(d e) -> d e", e=EPC
                ),
            )
            raT = const.tile([D_REL, EPC], F16)
            nc.vector.tensor_copy(raT[:], ra_i8[:])

            # obj.T in SBUF (for the node-model MLP)
            objT = const.tile([D_OBJ, n_obj], F16)
            for k in range(n_obj // P):
                ot = gp.tile([P, D_OBJ], F16, tag="objload")
                nc.sync.dma_start(ot[:], obj2d[k * P : (k + 1) * P, :])
                tp = psp.tile([D_OBJ, P], F16, tag="ps")
                nc.tensor.transpose(tp[:], ot[:], ident16[:])
                nc.scalar.copy(objT[:, k * P : (k + 1) * P], tp[:])

            # pinned accumulator: e_agg.T [64, n_obj] (4 PSUM banks)
            agg_ps = aggp.tile([D_EFF, n_obj], F32)

            # ---- edge phase ------------------------------------------------
            for g in range(n_groups):
                rrt = []
                b1T = sp.tile([P, EG], F16, tag="b1T")
                for t in range(T):
                    c = g * T + t
                    # one-hot receiver rows for the aggregation matmul
                    oh = sp.tile([P, n_obj], F16, tag="oh")
                    nc.vector.tensor_tensor(
                        out=oh[:],
                        in0=idxr_h[:, c : c + 1].to_broadcast([P, n_obj]),
                        in1=iota16[:],
                        op=ALU.is_equal,
                    )
                    rrt.append(oh)

                    orr_t = gp.tile([P, D_OBJ], F16, tag="gat")
                    nc.gpsimd.indirect_dma_start(
                        out=orr_t[:], out_offset=None, in_=obj2d,
                        in_offset=bass.IndirectOffsetOnAxis(
                            ap=idxr_i[:, c : c + 1], axis=0
                        ),
                    )
                    tp = psp.tile([D_OBJ, P], F16, tag="ps")
                    nc.tensor.transpose(tp[:], orr_t[:], ident16[:])
                    nc.scalar.copy(b1T[0:D_OBJ, t * P : (t + 1) * P], tp[:])

                    ors_t = gp.tile([P, D_OBJ], F16, tag="gat")
                    nc.gpsimd.indirect_dma_start(
                        out=ors_t[:], out_offset=None, in_=obj2d,
                        in_offset=bass.IndirectOffsetOnAxis(
                            ap=idxs_i[:, c : c + 1], axis=0
                        ),
                    )
                    tp2 = psp.tile([D_OBJ, P], F16, tag="ps")
                    nc.tensor.transpose(tp2[:], ors_t[:], ident16[:])
                    nc.scalar.copy(b1T[D_OBJ : 2 * D_OBJ, t * P : (t + 1) * P], tp2[:])

                # relation MLP, feature-major [features, EG]
                h1p = psp.tile([H_REL, EG], F32, tag="ps")
                nc.tensor.matmul(h1p[:], w1ab[:], b1T[:], start=True, stop=False)
                nc.tensor.matmul(
                    h1p[:], w1c[:], raT[:, g * EG : (g + 1) * EG],
                    start=False, stop=True,
                )
                h1T = sp.tile([H_REL, EG], F16, tag="hT")
                nc.scalar.activation(h1T[:], h1p[:], AF.Relu, bias=b1t[:])

                h2p = psp.tile([H_REL, EG], F32, tag="ps")
                nc.tensor.matmul(h2p[:], w2[:], h1T[:], start=True, stop=True)
                h2T = sp.tile([H_REL, EG], F16, tag="hT")
                nc.scalar.activation(h2T[:], h2p[:], AF.Relu, bias=b2t[:])

                h3p = psp.tile([H_REL, EG], F32, tag="ps")
                nc.tensor.matmul(h3p[:], w3[:], h2T[:], start=True, stop=True)
                h3T = sp.tile([H_REL, EG], F16, tag="hT")
                nc.scalar.activation(h3T[:], h3p[:], AF.Relu, bias=b3t[:])

                h4p = psp.tile([D_EFF, EG], F32, tag="ps")
                nc.tensor.matmul(h4p[:], w4[:], h3T[:], start=True, stop=True)
                eT = sp.tile([D_EFF, EG], F16, tag="eT")
                nc.scalar.activation(eT[:], h4p[:], AF.Relu, bias=b4t[:])

                # aggregate: e_agg.T += e_chunk.T @ one_hot(idx_r)_chunk
                for t in range(T):
                    ep = psp.tile([P, D_EFF], F16, tag="ps")
                    nc.tensor.transpose(
                        ep[:], eT[:, t * P : (t + 1) * P], ident16[:D_EFF, :D_EFF]
                    )
                    ec = ecp.tile([P, D_EFF], F16, tag="ec")
                    nc.scalar.copy(ec[:], ep[:])
                    first = g == 0 and t == 0
                    last = g == n_groups - 1 and t == T - 1
                    for q in range(n_obj // NQ):
                        nc.tensor.matmul(
                            agg_ps[:, q * NQ : (q + 1) * NQ],
                            ec[:],
                            rrt[t][:, q * NQ : (q + 1) * NQ],
                            start=first,
                            stop=last,
                        )

            # ---- all-reduce e_agg across cores -----------------------------
            eagg_sb = const.tile([D_EFF, n_obj], F32)
            nc.scalar.copy(eagg_sb[:], agg_ps[:])
            cc_in = dp.tile([D_EFF, n_obj], F32)
            cc_out = dp.tile([D_EFF, n_obj], F32)
            nc.sync.dma_start(cc_in[:], eagg_sb[:])
            if use_collective:
                nc.gpsimd.collective_compute(
                    "AllReduce",
                    ALU.add,
                    replica_groups=[list(range(n_cores))],
                    ins=[cc_in.opt()],
                    outs=[cc_out.opt()],
                )
            else:
                nc.sync.dma_start(cc_out[:], cc_in[:])
            eaggT = const.tile([D_EFF, n_obj], F32)
            nc.sync.dma_start(eaggT[:], cc_out[:])
            eaggT16 = const.tile([D_EFF, n_obj], F16)
            nc.vector.tensor_copy(eaggT16[:], eaggT[:])

            # ---- node phase (object MLP) -----------------------------------
            pTt = const.tile([D_OUT, n_obj], F16)
            for q in range(n_nq):
                sl = slice(q * NQ, (q + 1) * NQ)
                cp = psp.tile([H_OBJ, NQ], F32, tag="ps")
                nc.tensor.matmul(cp[:], ow1a[:], objT[:, sl], start=True, stop=False)
                nc.tensor.matmul(cp[:], ow1b[:], eaggT16[:, sl], start=False, stop=True)
                hT = sp.tile([H_OBJ, NQ], F16, tag="hT")
                nc.scalar.activation(hT[:], cp[:], AF.Relu, bias=ob1t[:])
                pp = psp.tile([D_OUT, NQ], F32, tag="ps")
                nc.tensor.matmul(pp[:], ow2[:], hT[:], start=True, stop=True)
                nc.scalar.activation(pTt[:, sl], pp[:], AF.Identity, bias=ob2t[:])
            nc.sync.dma_start(pT_d[:, :], pTt[:])

    nc.compile()
    return nc


class _Res:
    """Minimal stand-in for BassKernelResults (no trace support)."""

    def __init__(self, results):
        self.results = results
        self.exec_time_ns = None
        self.mean_exec_time_ns = None
        self.instructions_and_trace = None
        self.profile_json = None


def _make_runner(nc, n_cores):
    """Build the jitted shard_map executable ONCE; warm calls only pay
    input upload + execution (run_bass_via_pjrt re-creates the closure and
    re-traces on every call)."""
    import jax
    from jax.experimental.shard_map import shard_map
    from jax.sharding import Mesh, PartitionSpec

    from concourse.bass2jax import (
        _bass_exec_p,
        install_neuronx_cc_hook,
        partition_id_tensor,
    )

    install_neuronx_cc_hook()

    partition_name = nc.partition_id_tensor.name if nc.partition_id_tensor else None
    dbg_name = nc.dbg_addr.name if nc.dbg_addr is not None else None

    in_names = []
    out_names = []
    out_avals = []
    out_shapes = []
    for alloc in nc.m.functions[0].allocations:
        if not isinstance(alloc, mybir.MemoryLocationSet):
            continue
        name = alloc.memorylocations[0].name
        if alloc.kind == "ExternalInput":
            if name != partition_name:
                in_names.append(name)
        elif alloc.kind == "ExternalOutput":
            shape = tuple(alloc.tensor_shape)
            dtype = mybir.dt.np(alloc.dtype)
            out_names.append(name)
            out_avals.append(jax.core.ShapedArray(shape, dtype))
            out_shapes.append((shape, dtype))
    n_params = len(in_names)
    all_names = list(in_names) + list(out_names)
    if partition_name is not None:
        all_names.append(partition_name)

    donate = tuple(range(n_params, n_params + len(out_names)))

    def _body(*args):
        operands = list(args)
        if partition_name is not None:
            operands.append(partition_id_tensor())
        outs = _bass_exec_p.bind(
            *operands,
            out_avals=tuple(out_avals),
            in_names=tuple(all_names),
            out_names=tuple(out_names),
            lowering_input_output_aliases=(),
            sim_require_finite=True,
            sim_require_nnan=True,
            nc=nc,
        )
        return tuple(outs)

    devices = jax.devices()[:n_cores]
    assert len(devices) == n_cores
    mesh = Mesh(np.asarray(devices), ("core",))
    replicated = set()  # all inputs are per-core shards
    in_specs = tuple(
        PartitionSpec() if nm in replicated else PartitionSpec("core")
        for nm in in_names
    ) + (PartitionSpec("core"),) * len(out_names)
    out_specs = (PartitionSpec("core"),) * len(out_names)
    sharded = jax.jit(
        shard_map(
            _body, mesh=mesh, in_specs=in_specs, out_specs=out_specs,
            check_rep=False,
        ),
        donate_argnums=donate,
        keep_unused=True,
    )

    zeros_cache = [
        np.zeros((n_cores * shape[0], *shape[1:]), dtype)
        for shape, dtype in out_shapes
    ]
    dbg_zero = (
        np.zeros((n_cores, 2), np.uint32) if dbg_name is not None else None
    )

    prev_outs = [None] * len(out_names)

    def run(named_flats):
        """named_flats: dict input-name -> already-concatenated global array
        ([n_cores * per_core_len, ...])."""
        if dbg_name is not None:
            named_flats = {**named_flats, dbg_name: dbg_zero}
        concat_in = [named_flats[nm] for nm in in_names]
        # recycle the previous call's (fully overwritten) output buffers as
        # the donated output operands — skips the zeros upload on warm calls
        outs_in = [
            p if p is not None else z
            for p, z in zip(prev_outs, zeros_cache)
        ]
        out_arrs = sharded(*concat_in, *outs_in)
        for i, arr in enumerate(out_arrs):
            prev_outs[i] = arr
        # fetch only core 0's shard (all cores produce the full output)
        results0 = {}
        for i, name in enumerate(out_names):
            arr = out_arrs[i]
            try:
                shard0 = np.asarray(arr.addressable_shards[0].data)
                if shard0.shape != out_shapes[i][0]:
                    shard0 = shard0.reshape(n_cores, *out_shapes[i][0])[0]
            except Exception:
                shard0 = np.asarray(arr).reshape(n_cores, *out_shapes[i][0])[0]
            results0[name] = shard0
        return [results0]

    return run


_CACHE = {}
TRACE = False  # kept for test.py compat; tracing unsupported on this setup

_SCAN_SRC = r"""
#include <string.h>
#include <math.h>
/* One-hot rows contain exactly one 1.0f (bytes 00 00 80 3F) among 0.0f
   (all-zero bytes), so the first 0x3F byte sits at byte 4*idx+3. glibc's
   SIMD memchr with per-row early exit beats a full BLAS sgemv read. */
void onehot_memchr(const float* af, long n, long m, float* out) {
    const char* a = (const char*)af;
    long rowb = m * 4;
    for (long i = 0; i < n; i++) {
        const char* row = a + i * rowb;
        const char* p = (const char*)memchr(row, 0x3F, rowb);
        out[i] = p ? (float)((p - row - 3) >> 2) : 0.0f;
    }
}
/* One-pass int8 quantize + transpose + scatter of ra into the per-core
   blob regions: out[c][d*epc + e] = rint(ra[c*epc+e, d] * inv_s[d]).
   Blocked over 64 edges so reads stay in L1 and writes are 64B runs. */
void quant_ra(const float* ra, long n_cores, long epc, long d,
              const float* inv_s, char* out0, long core_stride) {
    for (long c = 0; c < n_cores; c++) {
        const float* rc = ra + c * epc * d;
        char* oc = out0 + c * core_stride;
        for (long e0 = 0; e0 < epc; e0 += 64) {
            for (long dd = 0; dd < d; dd++) {
                float s = inv_s[dd];
                char* op = oc + dd * epc + e0;
                const float* ip = rc + e0 * d + dd;
                for (long k = 0; k < 64; k++) {
                    op[k] = (char)lrintf(ip[k * d] * s);
                }
            }
        }
    }
}
"""


def _get_scanner():
    """Returns scan(a)->float32 indices for one-hot rows; C memchr fast
    path with a BLAS sgemv fallback."""
    if "scan" in _CACHE:
        return _CACHE["scan"]
    scan = None
    try:
        import importlib
        import tempfile

        import cffi

        ffi = cffi.FFI()
        ffi.cdef(
            "void onehot_memchr(const float* a, long n, long m, float* out);"
            "void quant_ra(const float* ra, long n_cores, long epc, long d,"
            "              const float* inv_s, char* out0, long core_stride);"
        )
        d = tempfile.mkdtemp()
        ffi.set_source("_onehot_scan_knl", _SCAN_SRC,
                       extra_compile_args=["-O3", "-march=native"])
        ffi.compile(tmpdir=d)
        if d not in sys.path:
            sys.path.insert(0, d)
        mod = importlib.import_module("_onehot_scan_knl")
        _CACHE["cmod"] = mod

        def scan(a):
            a = np.ascontiguousarray(a, dtype=np.float32)
            out = np.empty(a.shape[0], np.float32)
            mod.lib.onehot_memchr(
                mod.ffi.cast("float*", a.ctypes.data),
                a.shape[0], a.shape[1],
                mod.ffi.cast("float*", out.ctypes.data),
            )
            return out

        # self-check on a tiny case before trusting it
        chk = np.zeros((4, 8), np.float32)
        chk[[0, 1, 2, 3], [5, 0, 7, 3]] = 1.0
        assert np.array_equal(scan(chk), np.array([5, 0, 7, 3], np.float32))
    except Exception as e:
        print(f"kernel: memchr scanner unavailable ({e!r}); using BLAS",
              file=sys.stderr)

        def scan(a):
            a = np.asarray(a, dtype=np.float32)
            return a @ np.arange(a.shape[1], dtype=np.float32)

    _CACHE["scan"] = scan
    return scan


def _get_nc():
    if "nc" not in _CACHE:
        _CACHE["nc"] = build()
    return _CACHE["nc"]


def _pack_inputs(inputs):
    """Host-side marshalling: exact index extraction + packed f16 shards
    (preallocated buffers reused across calls)."""
    f32 = lambda k: np.asarray(inputs[k], dtype=np.float32)
    rr, rs, ra = f32("rr"), f32("rs"), f32("ra")
    obj = f32("obj")
    scan = _get_scanner()
    idx_r = scan(rr)  # exact: single 1.0 per row, values < 2^11
    idx_s = scan(rs)

    if "bufs" not in _CACHE:
        _CACHE["bufs"] = np.zeros((N_CORES, SSZ), np.float16)
    shard = _CACHE["bufs"]

    # int8-quantize ra per feature; fold the dequant scale into w1c so the
    # device MLP is unchanged: ra @ w1c == q @ (diag(s) @ w1c)
    ra = np.ascontiguousarray(ra)
    ra_s = np.maximum(np.abs(ra).max(axis=0) / 127.0, 1e-12).astype(np.float32)
    w1c_scaled = f32("rm_w1")[P : P + D_REL] * ra_s[:, None]

    w_flat = np.zeros(W_PAD, np.float16)
    pieces = [
        f32("rm_w1")[0:P], w1c_scaled, f32("rm_b1"),
        f32("rm_w2"), f32("rm_b2"), f32("rm_w3"), f32("rm_b3"),
        f32("rm_w4"), f32("rm_b4"),
        f32("om_w1")[0:D_OBJ], f32("om_w1")[D_OBJ : D_OBJ + D_EFF],
        f32("om_b1"), f32("om_w2"), f32("om_b2"),
    ]
    o = 0
    for p_ in pieces:
        n = p_.size
        w_flat[o : o + n] = p_.astype(np.float16).ravel()
        o += n
    assert o == W_TOTAL

    obj16 = obj.astype(np.float16).ravel()
    for c in range(N_CORES):
        shard[c, 0:OBJ_SH] = obj16[c * OBJ_SH : (c + 1) * OBJ_SH]
        shard[c, OBJ_SH : OBJ_SH + W_SH] = w_flat[c * W_SH : (c + 1) * W_SH]

    idx_r3 = idx_r.reshape(N_CORES, NCH, P)
    idx_s3 = idx_s.reshape(N_CORES, NCH, P)
    for c in range(N_CORES):
        shard[c, O_IR : O_IR + EPC] = idx_r3[c].T.astype(np.float16).ravel()
        shard[c, O_IS : O_IS + EPC] = idx_s3[c].T.astype(np.float16).ravel()

    mod = _CACHE.get("cmod")
    if mod is not None:
        inv_s = np.ascontiguousarray(1.0 / ra_s, dtype=np.float32)
        mod.lib.quant_ra(
            mod.ffi.cast("float*", ra.ctypes.data),
            N_CORES, EPC, D_REL,
            mod.ffi.cast("float*", inv_s.ctypes.data),
            mod.ffi.cast("char*", shard.ctypes.data) + 2 * O_RA,
            2 * SSZ,
        )
    else:
        ra_q = np.rint(ra / ra_s).astype(np.int8)
        for c in range(N_CORES):
            shard[c, O_RA : O_RA + RA_SLOTS].view(np.int8)[...] = (
                ra_q[c * EPC : (c + 1) * EPC].T.ravel()
            )
    return shard


def kernel(**inputs):
    nc = _get_nc()
    shard = _pack_inputs(inputs)

    if "runner" not in _CACHE:
        try:
            _CACHE["runner"] = _make_runner(nc, N_CORES)
        except Exception as e:
            print(f"kernel: cached runner unavailable ({e!r}); "
                  f"falling back to run_bass_kernel_spmd", file=sys.stderr)
            _CACHE["runner"] = None
    runner = _CACHE["runner"]
    if runner is not None:
        results = runner({"shard": shard.reshape(-1)})
        res = _Res(results)
    else:
        in_maps = [{"shard": shard[c]} for c in range(N_CORES)]
        res = run_bass_kernel_spmd(
            nc, in_maps, core_ids=list(range(N_CORES)), trace=False
        )
    _CACHE["last_results"] = res
    return np.ascontiguousarray(res.results[0]["pT"].T.astype(np.float32))
